# revision 57
# baseline (speedup 1.0000x reference)
"""Trainium2 Bass kernel for AttentionSR (spatial-reduction attention), v3.

Reference computation (per batch b):
  q = x @ Wq.T                                   [4096, 512] -> heads [8, 4096, 64]
  x_ = conv2x2_stride2(x as NCHW image, Wsr) + bsr   -> [1024, 512]
  x_ = layernorm(x_, g, b)
  k, v = split(x_ @ Wkv.T)                       [8, 1024, 64] each
  out = softmax(q k^T / 8) v                     -> [4096, 512]
  y = out @ Wp.T + bp
Sharding (8 cores): core = 2*batch + query_half. Each core owns one batch's
conv/LN/KV (duplicated across the pair) and 2048 of its 4096 query rows.
No collectives.

v3 design (from v2 trace analysis: attention phase was perfectly exp-paced
at 128 x 1114ns ACT instructions, but phase A ran 66us serial BEFORE the
first exp, and the attention window had ~55us of unused PE slack):
  - two-pass attention: pass 0 = kt 0-3 (kv half 0), pass 1 = kt 4-7.
    Between passes each unit's av psum is evacuated to a bf16 sbuf partial
    (DVE) and re-injected in pass 1 via an identity matmul (PE, cheap).
  - prefix before the first exp is only: conv half 0 -> stats0/LN0 ->
    kv half 0 -> q chunk 0.  Everything else (conv half 1, stats1/LN1,
    kv half 1, q chunks 1-3, proj) is spilled into the attention-phase PE
    slack with deprioritized emission, deadline-ordered.
  - spilled psum work uses a 2-bank spill pool; spilled evacs go to DVE
    (ACT is saturated by the exp stream); prefix evacs stay on ACT.
  - scores stay row-tiled (two concurrent K=64 matmuls, auto tile_position
    from base partitions 0/64); av full-array K=128; denominator via 64
    ones-columns in v (psum rows 0:64), evac = recip + mul on DVE.
  - tail: final proj evacs on ACT (idle after last exp).
"""

import numpy as np
import ml_dtypes
from contextlib import ExitStack

import concourse.bass as bass
import concourse.bacc as bacc
import concourse.tile as tile
from concourse import mybir
from concourse.bass_utils import run_bass_kernel_spmd

BF = ml_dtypes.bfloat16
F32 = mybir.dt.float32
F32R = mybir.dt.float32r
BF16 = mybir.dt.bfloat16
AF = mybir.ActivationFunctionType
ALU = mybir.AluOpType

C = 512          # model dim
NHEAD = 8
DH = 64          # head dim
HS = WS = 64     # image height/width
NTOK = HS * WS   # 4096 tokens per batch
NQ = 2048        # query rows per core
NKV = 1024       # reduced tokens (keys)
B = 4
SCALE = DH ** -0.5
EPS = 1e-5
# priority offset for attention-phase filler work (local: the scheduler
# treats priority mostly as program position, so spill must be emitted in
# small chunks near where it should run, nudged just past the local
# attention stream)
SPILL_PRIO = 16


def _emit(nc, tc, ctx, io):
    (xq, xo, w2, wq, wkg, wkg2, wvg, wvg2, wp, bsr_t, bp_t,
     ones_row, ones_c1, ident_in, yt) = io

    persist = ctx.enter_context(tc.tile_pool(name="persist", bufs=1))
    small = ctx.enter_context(tc.tile_pool(name="small", bufs=1))

    # ---- persistent sbuf tensors ----
    xh0 = persist.tile([128, 4, NQ], BF16, tag="xh0")
    # xh1 is dead after conv half 1 (mid-pass-0); vout is first written in
    # pass 1 -> vout overlays xh1's 16KB buffer (allocated at pass-1 start)
    pxh1 = ctx.enter_context(tc.tile_pool(name="pxh1", bufs=1))
    xh1 = pxh1.tile([128, 4, NQ], BF16, tag="xh1")
    # w2 is dead after conv half 1 (mid-pass-0); wp is first read by proj in
    # pass 1 -> overlay wp on w2_0's buffer (bufs=1 tag reuse)
    pw2 = ctx.enter_context(tc.tile_pool(name="pw2", bufs=1))
    w2_sb = [pw2.tile([128, 4, C], BF16, tag=f"w2_{i}", name=f"w2_{i}") for i in range(4)]
    wq_sb = persist.tile([128, 4, C], BF16, tag="wq")
    wkg_sb = persist.tile([128, 4, C], BF16, tag="wkg")
    wkg2_sb = persist.tile([2, C], BF16, tag="wkg2")
    wvg_sb = persist.tile([128, 4, C], BF16, tag="wvg")
    wvg2_sb = persist.tile([2, C], BF16, tag="wvg2")
    ident = persist.tile([128, 128], BF16, tag="ident")

    qT = [persist.tile([128, NQ], BF16, tag=f"qT{i}", name=f"qT{i}") for i in range(4)]
    kT0 = [persist.tile([128, 512], BF16, tag=f"kT0{i}", name=f"kT0{i}") for i in range(4)]
    kT1 = [persist.tile([128, 512], BF16, tag=f"kT1{i}", name=f"kT1{i}") for i in range(4)]
    # v with 64 ones-columns prepended per head: av matmul M=128, psum rows
    # 0..63 = softmax denominator (64x replicated)
    v_sb = [persist.tile([128, NHEAD, 2 * DH], BF16, tag=f"v{i}", name=f"v{i}")
            for i in range(8)]
    vout_holder = []

    def vout_ap(hp, qh):
        return vout_holder[0][:, 2 * hp + qh, :]
    # pass-0 partial av sums (per unit, [denominators+v for 2 heads])
    pv = [persist.tile([128, 1024], BF16, tag=f"pv{i}", name=f"pv{i}")
          for i in range(16)]
    # x_raw half-0 (prefix) and half-1 (mid-pass-0) lifetimes are disjoint:
    # rotate both through one 4-buffer pool
    pxr = ctx.enter_context(tc.tile_pool(name="pxr", bufs=1))
    x_raw = [[pxr.tile([128, 512], F32R, tag=f"xraw{i}",
                       name=f"xraw{h}_{i}") for i in range(4)]
             for h in range(2)]
    xs_ln = [[persist.tile([128, 512], BF16, tag=f"xsln{h}_{i}",
                           name=f"xsln{h}_{i}") for i in range(4)]
             for h in range(2)]
    xs_ext2 = [small.tile([2, 512], BF16, name=f"xs_ext2_{h}")
               for h in range(2)]          # row0 = -mu*rstd, row1 = ones (DMA)
    rstd_bc = [small.tile([128, 512], F32, name=f"rstd_bc_{h}")
               for h in range(2)]

    bsr_sb = small.tile([128, 4], F32)
    bp_sb = small.tile([128, 4], F32)
    ones_c = small.tile([128, 1], F32R)
    sm_row = [small.tile([1, 512], F32, name=f"sm_row{h}") for h in range(2)]
    mq_row = [small.tile([1, 512], F32, name=f"mq_row{h}") for h in range(2)]
    vr_row = [small.tile([1, 512], F32, name=f"vr_row{h}") for h in range(2)]
    rstd_row = [small.tile([1, 512], F32, name=f"rstd_row{h}") for h in range(2)]
    warm = small.tile([1, 8], F32)
    warm2 = small.tile([1, 8], F32)

    # ---------------- DMA in (interleaved so conv can start early) -------
    nc.sync.dma_start(out=xh0[:, 0, :], in_=xq[:, 0, :])
    for didj in range(4):
        nc.sync.dma_start(out=w2_sb[0][:, didj, :], in_=w2[0, :, didj, :])
    nc.sync.dma_start(out=xh0[:, 1, :], in_=xq[:, 1, :])
    nc.sync.dma_start(out=w2_sb[1][:], in_=w2[1])
    # warm the ACT exp table set under the DMA head
    nc.vector.memset(warm[:], 1.0)
    nc.scalar.activation(warm2[:], warm[:], AF.Exp)
    for ct in range(2, 4):
        nc.sync.dma_start(out=xh0[:, ct, :], in_=xq[:, ct, :])
        nc.sync.dma_start(out=w2_sb[ct][:], in_=w2[ct])
    nc.sync.dma_start(out=bsr_sb[:], in_=bsr_t)
    nc.sync.dma_start(out=wkg_sb[:], in_=wkg)
    nc.sync.dma_start(out=wkg2_sb[:], in_=wkg2)
    nc.sync.dma_start(out=wvg_sb[:], in_=wvg)
    nc.sync.dma_start(out=wvg2_sb[:], in_=wvg2)
    nc.sync.dma_start(out=wq_sb[:], in_=wq)
    for ct in range(4):
        nc.sync.dma_start(out=xh1[:, ct, :], in_=xo[:, ct, :])
    nc.sync.dma_start(out=bp_sb[:], in_=bp_t)
    nc.sync.dma_start(out=ones_c[:], in_=ones_c1)
    nc.sync.dma_start(out=ident[:], in_=ident_in)
    nc.sync.dma_start(out=xs_ext2[0][1:2, :], in_=ones_row[0:1, 0:512])
    nc.sync.dma_start(out=xs_ext2[1][1:2, :], in_=ones_row[0:1, 0:512])
    # wp overlays w2_0's buffer; allocated + DMA'd in the spill stream AFTER
    # conv half 1 is emitted (emission order defines dependency tracking)
    wp_holder = []
    # ones-columns FIRST: av psum rows 0..63 = softmax denominator
    for kt in range(8):
        nc.vector.memset(v_sb[kt][:, :, 0:DH], 1.0)
    wz = small.tile([128, 512], BF16)
    with tc.high_priority():
        nc.vector.memset(wz[:], 0.0)

    inv_c = 1.0 / C

    # one psum pool set for the whole kernel:
    #   pp   2 x [128,1024] f32  (scores)            = 4 banks
    #   pav  1 x [128,1024] f32  (av accumulators)   = 2 banks
    #   psp  2 x [128, 512] f32  (everything else)   = 2 banks
    pp = ctx.enter_context(tc.tile_pool(name="pp", bufs=2, space="PSUM"))
    pav = ctx.enter_context(tc.tile_pool(name="pav", bufs=1, space="PSUM"))
    psp = ctx.enter_context(tc.tile_pool(name="psp", bufs=2, space="PSUM"))
    pexp = ctx.enter_context(tc.tile_pool(name="pexp", bufs=4))
    prb = ctx.enter_context(tc.tile_pool(name="prb", bufs=2))
    pyb = ctx.enter_context(tc.tile_pool(name="pyb", bufs=1))

    pxsq = ctx.enter_context(tc.tile_pool(name="pxsq", bufs=4))

    # ================= phase-A building blocks ===========================
    def conv_half(half, xh, evac_dve):
        xsq = []
        for ot in range(4):
            ps = psp.tile([128, 512], F32, tag="sp", name=f"conv{half}_{ot}")
            psv = ps[:].rearrange("p (a b) -> p a b", a=16)
            for ct in range(4):
                for didj in range(4):
                    di, dj = didj // 2, didj % 2
                    rhs = bass.AP(
                        tensor=xh[:].tensor,
                        offset=xh[:].offset + ct * NQ + di * WS + dj,
                        ap=[xh[:].ap[0], [2 * WS, 16], [2, 32]],
                    )
                    nc.tensor.matmul(
                        psv, lhsT=w2_sb[ct][:, didj, ot * 128:(ot + 1) * 128],
                        rhs=rhs, start=(ct == 0 and didj == 0),
                        stop=(ct == 3 and didj == 3))
            if evac_dve:
                nc.vector.tensor_scalar_add(x_raw[half][ot][:],
                                            ps[:], bsr_sb[:, ot:ot + 1])
            else:
                nc.scalar.activation(x_raw[half][ot][:], ps[:], AF.Identity,
                                     bias=bsr_sb[:, ot:ot + 1])
            t = pxsq.tile([128, 512], F32R, tag="xsq", name="xsq")
            nc.vector.tensor_mul(t[:], x_raw[half][ot][:].bitcast(F32),
                                 x_raw[half][ot][:].bitcast(F32))
            xsq.append(t)
        return xsq

    def stats_half(half, xsq, hi_prio=True):
        ps = psp.tile([128, 512], F32, tag="sp", name=f"stx{half}")
        ps2 = psp.tile([128, 512], F32, tag="sp", name=f"stq{half}")

        def emit():
            for ct in range(4):
                nc.tensor.matmul(ps[0:1, :], lhsT=ones_c[:],
                                 rhs=x_raw[half][ct][:],
                                 start=(ct == 0), stop=(ct == 3))
            for ct in range(4):
                nc.tensor.matmul(ps2[0:1, :], lhsT=ones_c[:],
                                 rhs=xsq[ct][:],
                                 start=(ct == 0), stop=(ct == 3))
        if hi_prio:
            with tc.high_priority():
                emit()
        else:
            emit()
        return ps, ps2

    def ln_half_ops(half, ps, ps2, act_rsqrt=False):
        """LN rstd chain as thunks.  Each cross-op dependency hop costs
        ~570ns of semaphore latency, so the prefix (half 0) uses a single
        ACT Rsqrt (ScalarE idle there; table load hides under the DMA/conv
        head).  Half 1 runs mid-attention where the exp table must stay
        loaded, so it keeps the DVE Newton chain (2 iterations)."""
        sm, mq = sm_row[half][0:1, :], mq_row[half][0:1, :]
        vr, rs = vr_row[half][0:1, :], rstd_row[half][0:1, :]
        ops = []
        ops.append(lambda: nc.vector.tensor_scalar_mul(sm, ps[0:1, :], inv_c))
        ops.append(lambda: nc.vector.tensor_mul(mq, sm, sm))
        ops.append(lambda: nc.vector.scalar_tensor_tensor(
            vr, ps2[0:1, :], inv_c, mq, op0=ALU.mult, op1=ALU.subtract))
        ops.append(lambda: nc.vector.tensor_scalar_add(vr, vr, EPS))
        # rstd = exp(-0.5*ln(var+eps)): 2 ACT ops from the SAME table set
        # as the attention exp (natural_log_exp_and_others) -> no table
        # switch, and far fewer ~570ns dependency hops than a Newton chain
        ops.append(lambda: nc.scalar.activation(mq, vr, AF.Ln))
        ops.append(lambda: nc.scalar.activation(rs, mq, AF.Exp, scale=-0.5))
        ops.append(lambda: nc.gpsimd.partition_broadcast(
            rstd_bc[half][:], rs))
        for ct in range(4):
            ops.append(lambda ct=ct: nc.vector.tensor_mul(
                xs_ln[half][ct][:],
                x_raw[half][ct][:].bitcast(F32), rstd_bc[half][:]))
        ops.append(lambda: nc.vector.scalar_tensor_tensor(
            xs_ext2[half][0:1, :], sm, -1.0, rs,
            op0=ALU.mult, op1=ALU.mult))
        return ops

    def drain(ops, n):
        for _ in range(min(n, len(ops))):
            ops.pop(0)()

    def k_group(half, ot, evac_dve):
        kTh = kT0 if half == 0 else kT1
        ps = psp.tile([128, 512], F32, tag="sp", name="ps_k")
        for ct in range(4):
            nc.tensor.matmul(ps[:],
                             lhsT=wkg_sb[:, ct, ot * 128:(ot + 1) * 128],
                             rhs=xs_ln[half][ct][:],
                             start=(ct == 0), stop=False)
        nc.tensor.matmul(ps[:], lhsT=wkg2_sb[:, ot * 128:(ot + 1) * 128],
                         rhs=xs_ext2[half][:], start=False, stop=True)
        if evac_dve:
            nc.vector.tensor_copy(kTh[ot][:], ps[:])
        else:
            nc.scalar.copy(kTh[ot][:], ps[:])

    def v_group(half, tt, evac_dve):
        sl = slice((tt % 4) * 128, (tt % 4) * 128 + 128)
        ps = psp.tile([128, 512], F32, tag="sp", name="ps_v")
        for ct in range(4):
            nc.tensor.matmul(ps[:], lhsT=xs_ln[half][ct][:, sl],
                             rhs=wvg_sb[:, ct, :],
                             start=(ct == 0), stop=False)
        nc.tensor.matmul(ps[:], lhsT=xs_ext2[half][:, sl],
                         rhs=wvg2_sb[:], start=False, stop=True)
        src = ps[:].rearrange("p (h d) -> p h d", h=NHEAD)
        if evac_dve:
            nc.vector.tensor_copy(v_sb[tt][:, :, DH:2 * DH], src)
        else:
            nc.scalar.copy(v_sb[tt][:, :, DH:2 * DH], src)

    def q_group(ot, qc, evac_dve):
        ps = psp.tile([128, 512], F32, tag="sp", name="ps_q")
        for ct in range(4):
            nc.tensor.matmul(
                ps[:], lhsT=wq_sb[:, ct, ot * 128:(ot + 1) * 128],
                rhs=xh0[:, ct, qc * 512:(qc + 1) * 512],
                start=(ct == 0), stop=(ct == 3))
        if evac_dve:
            nc.vector.tensor_copy(qT[ot][:, qc * 512:(qc + 1) * 512], ps[:])
        else:
            nc.scalar.copy(qT[ot][:, qc * 512:(qc + 1) * 512], ps[:])

    proj_ps = {}

    def proj_part(qc, ot, part, evac_scalar=False):
        """Half of a proj chunk (2 matmuls): spilled mid-unit in pass 1 so
        it never displaces the next unit's scores at a boundary."""
        qh, qr = qc // 2, (qc % 2) * 512
        if part == 0:
            proj_ps[(qc, ot)] = psp.tile([128, 512], F32, tag="sp",
                                         name="ps_proj")
        ps = proj_ps[(qc, ot)]
        wp_sb = wp_holder[0]
        for ct in (0, 1) if part == 0 else (2, 3):
            nc.tensor.matmul(
                ps[:], lhsT=wp_sb[:, ct, ot * 128:(ot + 1) * 128],
                rhs=vout_ap(ct, qh)[:, qr:qr + 512],
                start=(ct == 0), stop=(ct == 3))
        if part == 1:
            yb = pyb.tile([128, 512], BF16, tag="yb", name="yb")
            if evac_scalar:
                nc.scalar.activation(yb[:], ps[:], AF.Identity,
                                     bias=bp_sb[:, ot:ot + 1])
            else:
                nc.vector.tensor_scalar_add(yb[:], ps[:],
                                            bp_sb[:, ot:ot + 1])
            nc.sync.dma_start(
                out=yt[ot * 128:(ot + 1) * 128, qc * 512:(qc + 1) * 512],
                in_=yb[:])

    def proj_chunk(qc, ot, evac_scalar=False):
        proj_part(qc, ot, 0, evac_scalar)
        proj_part(qc, ot, 1, evac_scalar)

    # ================= prefix ============================================
    with tc.high_priority():    # dummy matmuls under the DMA head: HAM warm
        pw = psp.tile([128, 512], F32, tag="sp", name="pe_warm")
        for i in range(12):
            nc.tensor.matmul(pw[:], lhsT=wz[0:128, 0:128],
                             rhs=wz[:], start=(i == 0), stop=(i == 11))

    xsq0 = conv_half(0, xh0, evac_dve=False)
    st0, st0b = stats_half(0, xsq0)
    ops0 = ln_half_ops(0, st0, st0b, act_rsqrt=True)
    drain(ops0, 99)
    # re-warm the exp table (Rsqrt evicted it); lands under kv0's PE work
    nc.scalar.activation(warm2[:], warm[:], AF.Exp)
    # prefix keeps ONLY what attention unit 0's first two slots need;
    # the rest of kv0/q0 goes to the head of the spill queue (the static
    # scheduler orders same-engine work by priority, so true interleaving
    # with the attention stream requires interleaved EMISSION)
    q_group(0, 0, evac_dve=False)   # q0-ot0: first attention unit's queries
    k_group(0, 0, evac_dve=False)
    v_group(0, 0, evac_dve=False)
    v_group(0, 1, evac_dve=False)

    # ================= spill stream (fills attention PE slack) ==========
    spill = []
    xsq1 = []
    st1 = []
    conv1_ps = {}

    def conv1_part(ot, ct):
        """One ct-slice (4 matmuls) of conv half 1's ot psum group.
        Uses the (prefix-idle) score pool so it can execute during the
        LN0 chain without waiting on kv0's psp rotation."""
        if ct == 0:
            conv1_ps[ot] = pp.tile([128, 512], F32, tag="sc",
                                   name=f"conv1_{ot}")
        ps = conv1_ps[ot]
        psv = ps[:].rearrange("p (a b) -> p a b", a=16)
        for didj in range(4):
            di, dj = didj // 2, didj % 2
            rhs = bass.AP(
                tensor=xh1[:].tensor,
                offset=xh1[:].offset + ct * NQ + di * WS + dj,
                ap=[xh1[:].ap[0], [2 * WS, 16], [2, 32]],
            )
            nc.tensor.matmul(
                psv, lhsT=w2_sb[ct][:, didj, ot * 128:(ot + 1) * 128],
                rhs=rhs, start=(ct == 0 and didj == 0),
                stop=(ct == 3 and didj == 3))
        if ct == 3:
            nc.vector.tensor_scalar_add(x_raw[1][ot][:],
                                        ps[:], bsr_sb[:, ot:ot + 1])
            t = pxsq.tile([128, 512], F32R, tag="xsq", name="xsq")
            nc.vector.tensor_mul(t[:], x_raw[1][ot][:].bitcast(F32),
                                 x_raw[1][ot][:].bitcast(F32))
            xsq1.append(t)

    # conv half 1 + stats1 + LN1 run as PREFIX-spill at +prio: the scheduler
    # slots their (dependency-free) matmuls into the PE idle under the LN0
    # chain and kv0's evac bubbles.  kv half 1 is then ready early in the
    # attention phase, far ahead of its pass-1 deadline.
    tc.cur_priority += SPILL_PRIO
    for ot in range(4):
        for ct in range(4):
            conv1_part(ot, ct)

    wp_sb_t = pw2.tile([128, 4, C], BF16, tag="w2_0", name="wp_sb")
    nc.sync.dma_start(out=wp_sb_t[:], in_=wp)
    wp_holder.append(wp_sb_t)
    tc.cur_priority -= SPILL_PRIO

    def stats1_emit():
        st1.extend(stats_half(1, xsq1, hi_prio=False))

    ln1_holder = []

    def ln1_chunk(a, b):
        if not ln1_holder:
            ln1_holder.append(ln_half_ops(1, st1[0], st1[1]))
        for op in ln1_holder[0][a:b]:
            op()

    # attention-phase spill queue, deadline-ordered (score for pass-0 unit
    # u is EMITTED at slot 4u-LEAD, so its kT0/qT writers must be drained
    # before that): kv0/q0 tails first, then kv half 1 (pass-1), q1/q2/q3
    spill.append(lambda: k_group(0, 1, evac_dve=True))
    spill.append(lambda: q_group(1, 0, evac_dve=True))
    spill.append(lambda: v_group(0, 2, evac_dve=True))
    spill.append(lambda: v_group(0, 3, evac_dve=True))
    spill.append(lambda: k_group(0, 2, evac_dve=True))
    spill.append(lambda: q_group(2, 0, evac_dve=True))
    spill.append(lambda: k_group(0, 3, evac_dve=True))
    spill.append(lambda: q_group(3, 0, evac_dve=True))
    spill += [lambda ot=ot: q_group(ot, 1, evac_dve=True) for ot in range(4)]
    spill.append(stats1_emit)
    spill.append(lambda: ln1_chunk(0, 7))
    spill.append(lambda: ln1_chunk(7, 99))
    spill.append(lambda: k_group(1, 0, evac_dve=True))
    spill.append(lambda: v_group(1, 4, evac_dve=True))
    spill.append(lambda: v_group(1, 5, evac_dve=True))
    spill.append(lambda: v_group(1, 6, evac_dve=True))
    spill.append(lambda: v_group(1, 7, evac_dve=True))
    spill.append(lambda: k_group(1, 1, evac_dve=True))
    spill.append(lambda: k_group(1, 2, evac_dve=True))
    spill.append(lambda: k_group(1, 3, evac_dve=True))
    spill += [lambda ot=ot: q_group(ot, 2, evac_dve=True) for ot in range(4)]
    spill += [lambda ot=ot: q_group(ot, 3, evac_dve=True) for ot in range(4)]

    def drain_spill(n):
        for _ in range(min(n, len(spill))):
            f = spill.pop(0)
            tc.cur_priority += SPILL_PRIO
            f()
            tc.cur_priority -= SPILL_PRIO

    # ================= attention (two passes) ============================
    units = [(hp, qc) for qc in range(4) for hp in range(4)]

    def score_exp(hp, qc, kt):
        kTh = kT0 if kt < 4 else kT1
        ksl = slice((kt % 4) * 128, (kt % 4) * 128 + 128)
        qsl = slice(qc * 512, (qc + 1) * 512)
        sc = pp.tile([128, 1024], F32, tag="sc", name="sc")
        for sub in range(2):
            rr = sub * 64
            nc.tensor.matmul(
                sc[:, sub * 512:(sub + 1) * 512],
                lhsT=kTh[hp][rr:rr + 64, ksl],
                rhs=qT[hp][rr:rr + 64, qsl],
                start=True, stop=True)
        ex = pexp.tile([128, 1024], BF16, tag="ex", name="ex")
        nc.scalar.activation(ex[:], sc[:], AF.Exp, scale=SCALE)
        return ex

    # software pipeline: score/exp leads the av stream by one slot
    seq = []
    for pi in range(2):
        for ui, (hp, qc) in enumerate(units):
            for kt in range(pi * 4, pi * 4 + 4):
                seq.append((pi, ui, kt))

    # score/exp leads the av stream by THREE slots: at unit boundaries the
    # next unit's first av matmul waits for the previous unit's psum evac
    # (DVE, ~3.3us with sem hops); the lead keeps that wait behind more
    # scores in the PE queue, so the exp stream never starves.
    LEAD = 3

    def emit_score(m):
        if seq[m][0] == 1 and seq[m - 1][0] == 0:
            drain_spill(99)   # correctness backstop: all spill writers must
            # be emitted before any pass-1 reader; then overlay vout on xh1
            vout_holder.append(pxh1.tile([128, 8, 1024], BF16, tag="xh1",
                                         name="vout_all"))
        pi, ui, kt = seq[m]
        return score_exp(units[ui][0], units[ui][1], kt)

    ex_q = {m: emit_score(m) for m in range(LEAD)}
    av_cur = {}
    for n, (pi, ui, kt) in enumerate(seq):
        hp, qc = units[ui]
        qh, qr = qc // 2, (qc % 2) * 512
        # drain BEFORE emitting the lead score: spilled writers (kT/qT/v)
        # must be emitted before any consumer score
        if pi == 0 and kt % 4 != 0:
            drain_spill(2 if n < 10 else 1)
        if n + LEAD < len(seq):
            ex_q[n + LEAD] = emit_score(n + LEAD)
        if kt % 4 == 0:
            av = pav.tile([128, 1024], F32, tag="av", name=f"av{pi}_{ui}")
            av_cur[ui] = av
            if pi == 1:   # re-inject pass-0 partials via identity matmul
                nc.tensor.matmul(av[:, 0:512], lhsT=ident[:],
                                 rhs=pv[ui][:, 0:512], start=True, stop=False)
                nc.tensor.matmul(av[:, 512:1024], lhsT=ident[:],
                                 rhs=pv[ui][:, 512:1024], start=True, stop=False)
        av = av_cur[ui]
        ex = ex_q.pop(n)
        first = (kt % 4 == 0) and pi == 0
        last = (kt % 4 == 3)
        nc.tensor.matmul(av[:, 0:512], lhsT=v_sb[kt][:, 2 * hp, :],
                         rhs=ex[:, 0:512], start=first, stop=last)
        nc.tensor.matmul(av[:, 512:1024], lhsT=v_sb[kt][:, 2 * hp + 1, :],
                         rhs=ex[:, 512:1024], start=first, stop=last)
        if last:
            tc.cur_priority -= 8   # evacs must beat queued spill DVE work
            if pi == 0:
                nc.vector.tensor_copy(pv[ui][:], av[:])
            else:
                for h in range(2):
                    rbc = prb.tile([64, 512], F32, tag="rbc", name="rbc")
                    nc.vector.reciprocal_approx_fast(
                        out=rbc[:], in_=av[0:64, h * 512:(h + 1) * 512])
                    nc.vector.tensor_mul(
                        vout_ap(hp, qh)[h * 64:(h + 1) * 64, qr:qr + 512],
                        av[64:128, h * 512:(h + 1) * 512], rbc[:])
            tc.cur_priority += 8
        # pass-1 spill: proj chunks
        if pi == 1:
            if qc > 0 and kt % 4 in (1, 2):
                # proj for the previous qc block: 2-matmul halves mid-unit
                tc.cur_priority += SPILL_PRIO
                proj_part(qc - 1, ui % 4, kt % 4 - 1)
                tc.cur_priority -= SPILL_PRIO
    drain_spill(99)
    for ot in range(4):
        proj_chunk(3, ot, evac_scalar=True)


_CACHE = {}


def _build():
    if "nc" in _CACHE:
        return _CACHE["nc"]
    nc = bacc.Bacc("TRN2", target_bir_lowering=False, debug=False, num_devices=8)
    io = (
        nc.dram_tensor("xq", [128, 4, NQ], BF16, kind="ExternalInput").ap(),
        nc.dram_tensor("xo", [128, 4, NQ], BF16, kind="ExternalInput").ap(),
        nc.dram_tensor("w2", [4, 128, 4, C], BF16, kind="ExternalInput").ap(),
        nc.dram_tensor("wq", [128, 4, C], BF16, kind="ExternalInput").ap(),
        nc.dram_tensor("wkg", [128, 4, C], BF16, kind="ExternalInput").ap(),
        nc.dram_tensor("wkg2", [2, C], BF16, kind="ExternalInput").ap(),
        nc.dram_tensor("wvg", [128, 4, C], BF16, kind="ExternalInput").ap(),
        nc.dram_tensor("wvg2", [2, C], BF16, kind="ExternalInput").ap(),
        nc.dram_tensor("wp", [128, 4, C], BF16, kind="ExternalInput").ap(),
        nc.dram_tensor("bsr_t", [128, 4], F32, kind="ExternalInput").ap(),
        nc.dram_tensor("bp_t", [128, 4], F32, kind="ExternalInput").ap(),
        nc.dram_tensor("ones_row", [1, NKV], BF16, kind="ExternalInput").ap(),
        nc.dram_tensor("ones_c1", [128, 1], F32R, kind="ExternalInput").ap(),
        nc.dram_tensor("ident_in", [128, 128], BF16, kind="ExternalInput").ap(),
        nc.dram_tensor("yt", [C, NQ], BF16, kind="ExternalOutput").ap(),
    )
    with tile.TileContext(nc) as tc, ExitStack() as ctx:
        _emit(nc, tc, ctx, io)
    nc.compile()
    _CACHE["nc"] = nc
    return nc


def _prep_inputs(x, Wq, Wkv, Wsr, bsr, ln_g, ln_b, Wp, bp):
    x = np.asarray(x, np.float32)
    Wq = np.asarray(Wq, np.float32)
    Wkv = np.asarray(Wkv, np.float32)
    Wsr = np.asarray(Wsr, np.float32)
    bsr = np.asarray(bsr, np.float32)
    ln_g = np.asarray(ln_g, np.float32)
    ln_b = np.asarray(ln_b, np.float32)
    Wp = np.asarray(Wp, np.float32)
    bp = np.asarray(bp, np.float32)

    def pct(a):
        return np.ascontiguousarray(a.reshape(4, 128, -1).transpose(1, 0, 2))

    # w2[ct][128(part=in-ch), didj, out-ch]
    w2 = np.ascontiguousarray(
        Wsr.transpose(2, 3, 1, 0).reshape(4, 4, 128, C).transpose(1, 2, 0, 3)
        .astype(BF))
    wq = pct(Wq.T.astype(BF))
    Wk, Wv = Wkv[:C], Wkv[C:]

    def ext(W):
        main = pct((W * ln_g[None, :]).T.astype(BF))                    # [p, ct, o]
        rows = np.stack([W @ ln_g, W @ ln_b]).astype(BF)                # [2, o]
        return main, np.ascontiguousarray(rows)

    wkg, wkg2 = ext(Wk)
    wvg, wvg2 = ext(Wv)
    wp = pct(Wp.T.astype(BF))
    bsr_t = np.ascontiguousarray(bsr.reshape(4, 128).T)
    bp_t = np.ascontiguousarray(bp.reshape(4, 128).T)

    shared = dict(w2=w2, wq=wq, wkg=wkg, wkg2=wkg2, wvg=wvg, wvg2=wvg2,
                  wp=wp, bsr_t=bsr_t, bp_t=bp_t,
                  ones_row=np.ones((1, NKV), BF),
                  ones_c1=np.ones((128, 1), np.float32),
                  ident_in=np.eye(128, dtype=BF))
    in_maps = []
    for core in range(8):
        b, half = core // 2, core % 2
        xT = x[b].T.astype(BF)                # [C, NTOK]
        m = dict(shared)
        m["xq"] = pct(xT[:, half * NQ:(half + 1) * NQ])
        m["xo"] = pct(xT[:, (1 - half) * NQ:(2 - half) * NQ])
        in_maps.append(m)
    return in_maps


def kernel(x, H, W, Wq, Wkv, Wsr, bsr, ln_g, ln_b, Wp, bp, _trace=False):
    nc = _build()
    in_maps = _prep_inputs(x, Wq, Wkv, Wsr, bsr, ln_g, ln_b, Wp, bp)
    res = run_bass_kernel_spmd(nc, in_maps, list(range(8)), trace=_trace)
    y = np.empty((B, NTOK, C), np.float32)
    for core in range(8):
        b, half = core // 2, core % 2
        y[b, half * NQ:(half + 1) * NQ, :] = \
            res.results[core]["yt"].astype(np.float32).T
    kernel._last_result = res
    return y


# revision 58
# speedup vs baseline: 1.1695x; 1.1695x over previous
"""Trainium2 Bass kernel for AttentionSR (spatial-reduction attention), v2.1.

Reference computation (per batch b):
  q = x @ Wq.T                                   [4096, 512] -> heads [8, 4096, 64]
  x_ = conv2x2_stride2(x as NCHW image, Wsr) + bsr   -> [1024, 512]
  x_ = layernorm(x_, g, b)
  k, v = split(x_ @ Wkv.T)                       [8, 1024, 64] each
  out = softmax(q k^T / 8) v                     -> [4096, 512]
  y = out @ Wp.T + bp

Sharding (8 cores): core = 2*batch + query_half. Each core owns one batch's
conv/LN/KV (duplicated across the pair) and 2048 of its 4096 query rows.
No collectives.

v2 design notes (attention phase is ACT(exp)-throughput-bound: 128 exp
instructions of [128,1024] ~= 136us; everything else hides under it):
  - channel-major layout throughout, no PE transposes.
  - conv is ct(outer)-ordered so it can start as soon as the first input
    channel-block DMA lands; x DMA is split per channel-block.
  - v2.1: LN rstd = Exp(-0.5*Ln(var+eps)) on the scalar engine (same
    natural-log/exp table set as the attention exp -> no table switch),
    replacing the DVE Newton chain whose ~570ns/hop semaphore latency
    made it ~10us of serial critical path.
  - phase A PE stream has no bubbles: conv0, stats0, conv1 (LN0 on ACT/DVE
    underneath), stats1, kv0 (LN1 underneath), kv1, q(first 512 cols).
  - attention: per (head-pair, q-512-chunk): 8x { score pair (row-packed
    64-row matmuls), exp, one av matmul per head }.  av stationary is
    [ones64 | v] -> M=128, so psum rows 0..63 hold the softmax
    denominator replicated 64x: evac is reciprocal[64,512] + multiply,
    no partition-broadcast, no 1-lane row ops.
  - av psums are [128,512] (1 bank); scores 2x[128,1024]; 1 spare bank
    cycles between deferred q-projection chunks (qc+1) and output
    projection chunks (qc-1), interleaved into the attention PE slack.
  - output y is written bf16.
"""

import numpy as np
import ml_dtypes
from contextlib import ExitStack

import concourse.bass as bass
import concourse.bacc as bacc
import concourse.tile as tile
from concourse import mybir
from concourse.bass_utils import run_bass_kernel_spmd

BF = ml_dtypes.bfloat16
F32 = mybir.dt.float32
F32R = mybir.dt.float32r
BF16 = mybir.dt.bfloat16
AF = mybir.ActivationFunctionType
ALU = mybir.AluOpType

C = 512          # model dim
NHEAD = 8
DH = 64          # head dim
HS = WS = 64     # image height/width
NTOK = HS * WS   # 4096 tokens per batch
NQ = 2048        # query rows per core
NKV = 1024       # reduced tokens (keys)
B = 4
SCALE = DH ** -0.5
EPS = 1e-5


def _emit(nc, tc, ctx, io):
    (xq, xo, w2, wq, wkg, wkg2, wvg, wvg2, wp, bsr_t, bp_t,
     ones_row, ones_c1, yt) = io

    persist = ctx.enter_context(tc.tile_pool(name="persist", bufs=1))
    small = ctx.enter_context(tc.tile_pool(name="small", bufs=1))

    # ---- persistent sbuf tensors ----
    xh0 = persist.tile([128, 4, NQ], BF16, tag="xh0")
    xh1 = persist.tile([128, 4, NQ], BF16, tag="xh1")
    w2_sb = [persist.tile([128, 4, C], BF16, tag=f"w2_{i}", name=f"w2_{i}") for i in range(4)]
    wq_sb = persist.tile([128, 4, C], BF16, tag="wq")
    wkg_sb = persist.tile([128, 4, C], BF16, tag="wkg")
    wkg2_sb = persist.tile([2, C], BF16, tag="wkg2")
    wvg_sb = persist.tile([128, 4, C], BF16, tag="wvg")
    wvg2_sb = persist.tile([2, C], BF16, tag="wvg2")
    wp_sb = persist.tile([128, 4, C], BF16, tag="wp")

    qT = [persist.tile([128, NQ], BF16, tag=f"qT{i}", name=f"qT{i}") for i in range(4)]
    kT0 = [persist.tile([128, 512], BF16, tag=f"kT0{i}", name=f"kT0{i}") for i in range(4)]
    kT1 = [persist.tile([128, 512], BF16, tag=f"kT1{i}", name=f"kT1{i}") for i in range(4)]
    # v with 64 ones-columns appended per head: av matmul M=128, psum rows
    # 64..127 hold the softmax denominator (64x replicated)
    v_sb = [persist.tile([128, NHEAD, 2 * DH], BF16, tag=f"v{i}", name=f"v{i}")
            for i in range(8)]
    vout = [[persist.tile([128, 1024], BF16, tag=f"vout{i}_{h}", name=f"vout{i}_{h}")
             for h in range(2)] for i in range(4)]
    # per-half tiles (separate tiles so half-0 readers don't pick up false
    # dependencies on half-1 writers through whole-tile tracking)
    x_raw = [[persist.tile([128, 512], F32R, tag=f"xraw{h}_{i}",
                           name=f"xraw{h}_{i}") for i in range(4)]
             for h in range(2)]
    xs_ln = [[persist.tile([128, 512], BF16, tag=f"xsln{h}_{i}",
                           name=f"xsln{h}_{i}") for i in range(4)]
             for h in range(2)]
    xs_ext2 = [small.tile([2, 512], BF16, name=f"xs_ext2_{h}")
               for h in range(2)]          # row0 = -mu*rstd, row1 = ones (DMA)
    rstd_bc = [small.tile([128, 512], F32, name=f"rstd_bc_{h}")
               for h in range(2)]

    bsr_sb = small.tile([128, 4], F32)
    bp_sb = small.tile([128, 4], F32)
    ones_c = small.tile([128, 1], F32R)
    # LN row tensors ([1, N] tiles, base partition 0)
    sm_row = [small.tile([1, 512], F32, name=f"sm_row{h}") for h in range(2)]
    mq_row = [small.tile([1, 512], F32, name=f"mq_row{h}") for h in range(2)]
    vr_row = [small.tile([1, 512], F32, name=f"vr_row{h}") for h in range(2)]
    rstd_row = [small.tile([1, 512], F32, name=f"rstd_row{h}") for h in range(2)]
    warm = small.tile([1, 8], F32)
    warm2 = small.tile([1, 8], F32)
    warm3 = small.tile([1, 8], F32)

    # ---------------- DMA in (interleaved so conv can start early) -------
    nc.sync.dma_start(out=xh0[:, 0, :], in_=xq[:, 0, :])
    for didj in range(4):
        nc.sync.dma_start(out=w2_sb[0][:, didj, :], in_=w2[0, :, didj, :])
    nc.sync.dma_start(out=xh0[:, 1, :], in_=xq[:, 1, :])
    nc.sync.dma_start(out=w2_sb[1][:], in_=w2[1])
    # warm the ACT natural_log/exp table set under the DMA head
    nc.vector.memset(warm[:], 1.0)
    nc.scalar.activation(warm2[:], warm[:], AF.Exp)
    nc.scalar.activation(warm3[:], warm[:], AF.Ln)
    for ct in range(2, 4):
        nc.sync.dma_start(out=xh0[:, ct, :], in_=xq[:, ct, :])
        nc.sync.dma_start(out=w2_sb[ct][:], in_=w2[ct])
    nc.sync.dma_start(out=bsr_sb[:], in_=bsr_t)
    for ct in range(4):
        nc.sync.dma_start(out=xh1[:, ct, :], in_=xo[:, ct, :])
    nc.sync.dma_start(out=wkg_sb[:], in_=wkg)
    nc.sync.dma_start(out=wkg2_sb[:], in_=wkg2)
    nc.sync.dma_start(out=wvg_sb[:], in_=wvg)
    nc.sync.dma_start(out=wvg2_sb[:], in_=wvg2)
    nc.sync.dma_start(out=wq_sb[:], in_=wq)
    nc.sync.dma_start(out=wp_sb[:], in_=wp)
    nc.sync.dma_start(out=bp_sb[:], in_=bp_t)
    nc.sync.dma_start(out=ones_c[:], in_=ones_c1)
    nc.sync.dma_start(out=xs_ext2[0][1:2, :], in_=ones_row[0:1, 0:512])
    nc.sync.dma_start(out=xs_ext2[1][1:2, :], in_=ones_row[0:1, 0:512])
    # ones-columns FIRST: av psum rows 0..63 = softmax denominator (base
    # partition 0, so the reciprocal custom-op operands stay base-aligned)
    for kt in range(8):
        nc.vector.memset(v_sb[kt][:, :, 0:DH], 1.0)
    wz = small.tile([128, 512], BF16)
    with tc.high_priority():
        nc.vector.memset(wz[:], 0.0)

    inv_c = 1.0 / C

    # ================= Phase A: conv -> LN -> KV -> q(chunk0) ============
    with tc.tile_pool(name="ppa", bufs=4, space="PSUM") as ppa, \
         tc.tile_pool(name="px", bufs=2, space="PSUM") as px, \
         tc.tile_pool(name="pxsq", bufs=4) as pxsq:

        def conv_half(half, xh):
            hsl = slice(half * 512, (half + 1) * 512)
            # one psum tile per output-channel block: write-after-read deps
            # stay per-tile, so ot+1 matmuls don't wait on ot's evac
            pss = [ppa.tile([128, 512], F32, tag="conv", name=f"conv{half}_{ot}")
                   for ot in range(4)]
            psv = [pss[ot][:].rearrange("p (a b) -> p a b", a=16)
                   for ot in range(4)]

            def mm(ct, didj, ot, start, stop):
                di, dj = didj // 2, didj % 2
                rhs = bass.AP(
                    tensor=xh[:].tensor,
                    offset=xh[:].offset + ct * NQ + di * WS + dj,
                    ap=[xh[:].ap[0], [2 * WS, 16], [2, 32]],
                )
                nc.tensor.matmul(
                    psv[ot], lhsT=w2_sb[ct][:, didj, ot * 128:(ot + 1) * 128],
                    rhs=rhs, start=start, stop=stop)

            for ct in range(3):
                for didj in range(4):
                    for ot in range(4):
                        mm(ct, didj, ot, start=(ct == 0 and didj == 0), stop=False)
            xsq = []
            for ot in range(4):   # last ct pass ot-major so evacs pipeline
                for didj in range(4):
                    mm(3, didj, ot, start=False, stop=(didj == 3))
                nc.scalar.activation(x_raw[half][ot][:], pss[ot][:], AF.Identity,
                                     bias=bsr_sb[:, ot:ot + 1])
                t = pxsq.tile([128, 512], F32R, tag="xsq", name="xsq")
                nc.vector.tensor_mul(t[:], x_raw[half][ot][:].bitcast(F32),
                                     x_raw[half][ot][:].bitcast(F32))
                xsq.append(t)
            return xsq

        def stats_half(half, xsq):
            ps = px.tile([128, 1024], F32, tag="st", name=f"st{half}")
            with tc.high_priority():   # schedule as soon as x_raw/xsq land,
                for ct in range(4):    # not after the other half's conv
                    nc.tensor.matmul(ps[0:1, 0:512], lhsT=ones_c[:],
                                     rhs=x_raw[half][ct][:],
                                     start=(ct == 0), stop=(ct == 3))
                for ct in range(4):
                    nc.tensor.matmul(ps[0:1, 512:1024], lhsT=ones_c[:],
                                     rhs=xsq[ct][:],
                                     start=(ct == 0), stop=(ct == 3))
            return ps

        def ln_half_ops(half, ps):
            """LN rstd chain as a list of thunks so the emitter can
            interleave them between kv psum-groups.  rstd is computed as
            exp(-0.5*ln(var+eps)) on the scalar engine: the ln/exp pair
            shares one ACT table set with the attention exp, and replaces
            a DVE Newton chain whose ~570ns/hop semaphore latency put
            ~10us on the phase-A critical path."""
            sm, mq = sm_row[half][0:1, :], mq_row[half][0:1, :]
            vr, rs = vr_row[half][0:1, :], rstd_row[half][0:1, :]
            ops = []
            ops.append(lambda: nc.vector.tensor_scalar_mul(
                sm, ps[0:1, 0:512], inv_c))
            ops.append(lambda: nc.vector.tensor_mul(mq, sm, sm))
            ops.append(lambda: nc.vector.scalar_tensor_tensor(
                vr, ps[0:1, 512:1024], inv_c, mq,
                op0=ALU.mult, op1=ALU.subtract))
            ops.append(lambda: nc.vector.tensor_scalar_add(vr, vr, EPS))
            ops.append(lambda: nc.scalar.activation(mq, vr, AF.Ln))
            ops.append(lambda: nc.scalar.activation(rs, mq, AF.Exp,
                                                    scale=-0.5))
            ops.append(lambda: nc.gpsimd.partition_broadcast(
                rstd_bc[half][:], rs))
            for ct in range(4):
                ops.append(lambda ct=ct: nc.vector.tensor_mul(
                    xs_ln[half][ct][:],
                    x_raw[half][ct][:].bitcast(F32), rstd_bc[half][:]))
            ops.append(lambda: nc.vector.scalar_tensor_tensor(
                xs_ext2[half][0:1, :], sm, -1.0, rs,
                op0=ALU.mult, op1=ALU.mult))
            return ops

        def drain(ops, n):
            for _ in range(min(n, len(ops))):
                ops.pop(0)()

        def kv_half(half, interleave=None):
            # psum evacs go to the (phase-A idle) scalar engine so the DVE
            # FIFO stays free for the interleaved LN chain
            kTh = kT0 if half == 0 else kT1
            for ot in range(4):
                ps = px.tile([128, 1024], F32, tag="st", name="ps_k")
                for ct in range(4):
                    nc.tensor.matmul(ps[:, 0:512],
                                     lhsT=wkg_sb[:, ct, ot * 128:(ot + 1) * 128],
                                     rhs=xs_ln[half][ct][:],
                                     start=(ct == 0), stop=False)
                nc.tensor.matmul(ps[:, 0:512],
                                 lhsT=wkg2_sb[:, ot * 128:(ot + 1) * 128],
                                 rhs=xs_ext2[half][:], start=False, stop=True)
                nc.scalar.copy(kTh[ot][:], ps[:, 0:512])
                if interleave:
                    drain(interleave, 2)
            for tt in range(half * 4, half * 4 + 4):
                sl = slice((tt % 4) * 128, (tt % 4) * 128 + 128)
                ps = px.tile([128, 1024], F32, tag="st", name="ps_v")
                for ct in range(4):
                    nc.tensor.matmul(ps[:, 0:512], lhsT=xs_ln[half][ct][:, sl],
                                     rhs=wvg_sb[:, ct, :],
                                     start=(ct == 0), stop=False)
                nc.tensor.matmul(ps[:, 0:512], lhsT=xs_ext2[half][:, sl],
                                 rhs=wvg2_sb[:], start=False, stop=True)
                nc.scalar.copy(
                    v_sb[tt][:, :, DH:2 * DH],
                    ps[:, 0:512].rearrange("p (h d) -> p h d", h=NHEAD))
                if interleave:
                    drain(interleave, 3)

        def q_chunk_px(ot, qc):
            ps = px.tile([128, 1024], F32, tag="st", name="ps_q")
            for ct in range(4):
                nc.tensor.matmul(
                    ps[:, 0:512],
                    lhsT=wq_sb[:, ct, ot * 128:(ot + 1) * 128],
                    rhs=xh0[:, ct, qc * 512:(qc + 1) * 512],
                    start=(ct == 0), stop=(ct == 3))
            nc.scalar.copy(qT[ot][:, qc * 512:(qc + 1) * 512],
                           ps[:, 0:512])

        with tc.high_priority():    # ~3.5us of dummy matmuls under the DMA
            pw = px.tile([128, 1024], F32, tag="st", name="pe_warm")
            for i in range(16):
                nc.tensor.matmul(pw[:, 0:512], lhsT=wz[0:128, 0:128],
                                 rhs=wz[:], start=(i == 0), stop=(i == 15))
        xsq0 = conv_half(0, xh0)
        st0 = stats_half(0, xsq0)
        ops0 = ln_half_ops(0, st0)
        drain(ops0, 5)             # stats0-dependent head runs under conv1
        xsq1 = conv_half(1, xh1)
        drain(ops0, 99)
        st1 = stats_half(1, xsq1)
        ops1 = ln_half_ops(1, st1)
        for qc in range(2):        # q chunks are dep-free: Tile slots them
            for ot in range(4):    # into the PE idle under the LN chains
                q_chunk_px(ot, qc)
        kv_half(0, interleave=ops1)   # LN1 chain interleaves with kv0 evacs
        drain(ops1, 99)
        kv_half(1)

    # ================= Phase B: attention ================================
    with tc.tile_pool(name="pp", bufs=2, space="PSUM") as pp, \
         tc.tile_pool(name="pav", bufs=4, space="PSUM") as pav, \
         tc.tile_pool(name="pexp", bufs=6) as pexp, \
         tc.tile_pool(name="prb", bufs=2) as prb, \
         tc.tile_pool(name="pyb", bufs=2) as pyb:

        def q_chunk(ot, qc):
            ps = pav.tile([128, 512], F32, tag="av", name="ps_qd")
            for ct in range(4):
                nc.tensor.matmul(
                    ps[:], lhsT=wq_sb[:, ct, ot * 128:(ot + 1) * 128],
                    rhs=xh0[:, ct, qc * 512:(qc + 1) * 512],
                    start=(ct == 0), stop=(ct == 3))
            nc.vector.tensor_copy(qT[ot][:, qc * 512:(qc + 1) * 512], ps[:])

        def proj_chunk(qc, ot, evac_scalar=False):
            qh, qr = qc // 2, (qc % 2) * 512
            ps = pav.tile([128, 512], F32, tag="av", name="ps_proj")
            for ct in range(4):
                nc.tensor.matmul(
                    ps[:], lhsT=wp_sb[:, ct, ot * 128:(ot + 1) * 128],
                    rhs=vout[ct][qh][:, qr:qr + 512],
                    start=(ct == 0), stop=(ct == 3))
            yb = pyb.tile([128, 512], BF16, tag="yb", name="yb")
            if evac_scalar:   # tail: ACT is idle after the last exp
                nc.scalar.activation(yb[:], ps[:], AF.Identity,
                                     bias=bp_sb[:, ot:ot + 1])
            else:
                nc.vector.tensor_scalar_add(yb[:], ps[:], bp_sb[:, ot:ot + 1])
            nc.sync.dma_start(
                out=yt[ot * 128:(ot + 1) * 128, qc * 512:(qc + 1) * 512],
                in_=yb[:])

        def score_pair(hp, qc, kt):
            kTh = kT0 if kt < 4 else kT1
            ksl = slice((kt % 4) * 128, (kt % 4) * 128 + 128)
            qsl = slice(qc * 512, (qc + 1) * 512)
            sc = pp.tile([128, 1024], F32, tag="sc", name="sc")
            for sub in range(2):
                rr = sub * 64
                nc.tensor.matmul(
                    sc[:, sub * 512:(sub + 1) * 512],
                    lhsT=kTh[hp][rr:rr + 64, ksl],
                    rhs=qT[hp][rr:rr + 64, qsl],
                    start=True, stop=True)
            ex = pexp.tile([128, 1024], BF16, tag="ex", name="ex")
            nc.scalar.activation(ex[:], sc[:], AF.Exp, scale=SCALE)
            return ex

        # global software pipeline: the score/exp stream leads the av
        # stream by one kt slot and runs continuously ACROSS unit
        # boundaries, so the exp engine never waits at a boundary
        units = [(hp, qc) for qc in range(4) for hp in range(4)]
        blocks = [(i, 0, 8) for i in range(16)]
        seq = [(i, kt) for (i, b, e) in blocks for kt in range(b, e)]
        av_tiles = {}
        ex_p = score_pair(units[seq[0][0]][0], units[seq[0][0]][1], seq[0][1])
        for n, (idx, kt) in enumerate(seq):
            hp, qc = units[idx]
            qh, qr = qc // 2, (qc % 2) * 512
            if kt == 0:
                av_tiles[idx] = (
                    pav.tile([128, 512], F32, tag="av", name="av0"),
                    pav.tile([128, 512], F32, tag="av", name="av1"))
            av0, av1 = av_tiles[idx]
            ex = ex_p
            if n + 1 < len(seq):
                ni, nkt = seq[n + 1]
                ex_p = score_pair(units[ni][0], units[ni][1], nkt)
            nc.tensor.matmul(av0[:], lhsT=v_sb[kt][:, 2 * hp, :],
                             rhs=ex[:, 0:512],
                             start=(kt == 0), stop=(kt == 7))
            nc.tensor.matmul(av1[:], lhsT=v_sb[kt][:, 2 * hp + 1, :],
                             rhs=ex[:, 512:1024],
                             start=(kt == 0), stop=(kt == 7))
            if kt == 2 and qc in (1, 2):
                tc.cur_priority += 14   # filler work: schedule after the
                q_chunk(hp, qc + 1)     # score/exp stream
                tc.cur_priority -= 14
            if kt == 5 and qc > 0:
                tc.cur_priority += 14
                proj_chunk(qc - 1, hp)
                tc.cur_priority -= 14
            if kt == 7:
                for h, av in ((0, av0), (1, av1)):
                    rbc = prb.tile([64, 512], F32, tag="rbc", name="rbc")
                    nc.vector.reciprocal_approx_fast(out=rbc[:],
                                                     in_=av[0:64, :])
                    nc.vector.tensor_mul(
                        vout[hp][qh][h * 64:(h + 1) * 64, qr:qr + 512],
                        av[64:128, :], rbc[:])
        for ot in range(4):
            proj_chunk(3, ot, evac_scalar=True)


_CACHE = {}


def _build():
    if "nc" in _CACHE:
        return _CACHE["nc"]
    nc = bacc.Bacc("TRN2", target_bir_lowering=False, debug=False, num_devices=8)
    io = (
        nc.dram_tensor("xq", [128, 4, NQ], BF16, kind="ExternalInput").ap(),
        nc.dram_tensor("xo", [128, 4, NQ], BF16, kind="ExternalInput").ap(),
        nc.dram_tensor("w2", [4, 128, 4, C], BF16, kind="ExternalInput").ap(),
        nc.dram_tensor("wq", [128, 4, C], BF16, kind="ExternalInput").ap(),
        nc.dram_tensor("wkg", [128, 4, C], BF16, kind="ExternalInput").ap(),
        nc.dram_tensor("wkg2", [2, C], BF16, kind="ExternalInput").ap(),
        nc.dram_tensor("wvg", [128, 4, C], BF16, kind="ExternalInput").ap(),
        nc.dram_tensor("wvg2", [2, C], BF16, kind="ExternalInput").ap(),
        nc.dram_tensor("wp", [128, 4, C], BF16, kind="ExternalInput").ap(),
        nc.dram_tensor("bsr_t", [128, 4], F32, kind="ExternalInput").ap(),
        nc.dram_tensor("bp_t", [128, 4], F32, kind="ExternalInput").ap(),
        nc.dram_tensor("ones_row", [1, NKV], BF16, kind="ExternalInput").ap(),
        nc.dram_tensor("ones_c1", [128, 1], F32R, kind="ExternalInput").ap(),
        nc.dram_tensor("yt", [C, NQ], BF16, kind="ExternalOutput").ap(),
    )
    with tile.TileContext(nc) as tc, ExitStack() as ctx:
        _emit(nc, tc, ctx, io)
    nc.compile()
    _CACHE["nc"] = nc
    return nc


def _prep_inputs(x, Wq, Wkv, Wsr, bsr, ln_g, ln_b, Wp, bp):
    x = np.asarray(x, np.float32)
    Wq = np.asarray(Wq, np.float32)
    Wkv = np.asarray(Wkv, np.float32)
    Wsr = np.asarray(Wsr, np.float32)
    bsr = np.asarray(bsr, np.float32)
    ln_g = np.asarray(ln_g, np.float32)
    ln_b = np.asarray(ln_b, np.float32)
    Wp = np.asarray(Wp, np.float32)
    bp = np.asarray(bp, np.float32)

    def pct(a):
        return np.ascontiguousarray(a.reshape(4, 128, -1).transpose(1, 0, 2))

    # w2[ct][128(part=in-ch), didj, out-ch]
    w2 = np.ascontiguousarray(
        Wsr.transpose(2, 3, 1, 0).reshape(4, 4, 128, C).transpose(1, 2, 0, 3)
        .astype(BF))
    wq = pct(Wq.T.astype(BF))
    Wk, Wv = Wkv[:C], Wkv[C:]

    def ext(W):
        main = pct((W * ln_g[None, :]).T.astype(BF))                    # [p, ct, o]
        rows = np.stack([W @ ln_g, W @ ln_b]).astype(BF)                # [2, o]
        return main, np.ascontiguousarray(rows)

    wkg, wkg2 = ext(Wk)
    wvg, wvg2 = ext(Wv)
    wp = pct(Wp.T.astype(BF))
    bsr_t = np.ascontiguousarray(bsr.reshape(4, 128).T)
    bp_t = np.ascontiguousarray(bp.reshape(4, 128).T)

    shared = dict(w2=w2, wq=wq, wkg=wkg, wkg2=wkg2, wvg=wvg, wvg2=wvg2,
                  wp=wp, bsr_t=bsr_t, bp_t=bp_t,
                  ones_row=np.ones((1, NKV), BF),
                  ones_c1=np.ones((128, 1), np.float32))
    in_maps = []
    for core in range(8):
        b, half = core // 2, core % 2
        xT = x[b].T.astype(BF)                # [C, NTOK]
        m = dict(shared)
        m["xq"] = pct(xT[:, half * NQ:(half + 1) * NQ])
        m["xo"] = pct(xT[:, (1 - half) * NQ:(2 - half) * NQ])
        in_maps.append(m)
    return in_maps


def kernel(x, H, W, Wq, Wkv, Wsr, bsr, ln_g, ln_b, Wp, bp, _trace=False):
    nc = _build()
    in_maps = _prep_inputs(x, Wq, Wkv, Wsr, bsr, ln_g, ln_b, Wp, bp)
    res = run_bass_kernel_spmd(nc, in_maps, list(range(8)), trace=_trace)
    y = np.empty((B, NTOK, C), np.float32)
    for core in range(8):
        b, half = core // 2, core % 2
        y[b, half * NQ:(half + 1) * NQ, :] = \
            res.results[core]["yt"].astype(np.float32).T
    kernel._last_result = res
    return y


# revision 60
# speedup vs baseline: 1.1801x; 1.0091x over previous
"""Trainium2 Bass kernel for AttentionSR (spatial-reduction attention), v2.1.

Reference computation (per batch b):
  q = x @ Wq.T                                   [4096, 512] -> heads [8, 4096, 64]
  x_ = conv2x2_stride2(x as NCHW image, Wsr) + bsr   -> [1024, 512]
  x_ = layernorm(x_, g, b)
  k, v = split(x_ @ Wkv.T)                       [8, 1024, 64] each
  out = softmax(q k^T / 8) v                     -> [4096, 512]
  y = out @ Wp.T + bp

Sharding (8 cores): core = 2*batch + query_half. Each core owns one batch's
conv/LN/KV (duplicated across the pair) and 2048 of its 4096 query rows.
No collectives.

v2 design notes (attention phase is ACT(exp)-throughput-bound: 128 exp
instructions of [128,1024] ~= 136us; everything else hides under it):
  - channel-major layout throughout, no PE transposes.
  - conv is ct(outer)-ordered so it can start as soon as the first input
    channel-block DMA lands; x DMA is split per channel-block.
  - v2.1: LN rstd = Exp(-0.5*Ln(var+eps)) on the scalar engine (same
    natural-log/exp table set as the attention exp -> no table switch),
    replacing the DVE Newton chain whose ~570ns/hop semaphore latency
    made it ~10us of serial critical path.
  - phase A PE stream has no bubbles: conv0, stats0, conv1 (LN0 on ACT/DVE
    underneath), stats1, kv0 (LN1 underneath), kv1, q(first 512 cols).
  - attention: per (head-pair, q-512-chunk): 8x { score pair (row-packed
    64-row matmuls), exp, one av matmul per head }.  av stationary is
    [ones64 | v] -> M=128, so psum rows 0..63 hold the softmax
    denominator replicated 64x: evac is reciprocal[64,512] + multiply,
    no partition-broadcast, no 1-lane row ops.
  - av psums are [128,512] (1 bank); scores 2x[128,1024]; 1 spare bank
    cycles between deferred q-projection chunks (qc+1) and output
    projection chunks (qc-1), interleaved into the attention PE slack.
  - output y is written bf16.
"""

import numpy as np
import ml_dtypes
from contextlib import ExitStack

import concourse.bass as bass
import concourse.bacc as bacc
import concourse.tile as tile
from concourse import mybir
from concourse.bass_utils import run_bass_kernel_spmd

BF = ml_dtypes.bfloat16
F32 = mybir.dt.float32
F32R = mybir.dt.float32r
BF16 = mybir.dt.bfloat16
AF = mybir.ActivationFunctionType
ALU = mybir.AluOpType

C = 512          # model dim
NHEAD = 8
DH = 64          # head dim
HS = WS = 64     # image height/width
NTOK = HS * WS   # 4096 tokens per batch
NQ = 2048        # query rows per core
NKV = 1024       # reduced tokens (keys)
B = 4
SCALE = DH ** -0.5
EPS = 1e-5


def _emit(nc, tc, ctx, io):
    (xq, xo, w2, wq, wkg, wkg2, wvg, wvg2, wp, bsr_t, bp_t,
     ones_row, ones_c1, yt) = io

    persist = ctx.enter_context(tc.tile_pool(name="persist", bufs=1))
    small = ctx.enter_context(tc.tile_pool(name="small", bufs=1))

    # ---- persistent sbuf tensors ----
    xh0 = persist.tile([128, 4, NQ], BF16, tag="xh0")
    xh1 = persist.tile([128, 4, NQ], BF16, tag="xh1")
    w2_sb = [persist.tile([128, 4, C], BF16, tag=f"w2_{i}", name=f"w2_{i}") for i in range(4)]
    wq_sb = persist.tile([128, 4, C], BF16, tag="wq")
    wkg_sb = persist.tile([128, 4, C], BF16, tag="wkg")
    wkg2_sb = persist.tile([2, C], BF16, tag="wkg2")
    wvg_sb = persist.tile([128, 4, C], BF16, tag="wvg")
    wvg2_sb = persist.tile([2, C], BF16, tag="wvg2")
    wp_sb = persist.tile([128, 4, C], BF16, tag="wp")

    qT = [persist.tile([128, NQ], BF16, tag=f"qT{i}", name=f"qT{i}") for i in range(4)]
    kT0 = [persist.tile([128, 512], BF16, tag=f"kT0{i}", name=f"kT0{i}") for i in range(4)]
    kT1 = [persist.tile([128, 512], BF16, tag=f"kT1{i}", name=f"kT1{i}") for i in range(4)]
    # v with 64 ones-columns appended per head: av matmul M=128, psum rows
    # 64..127 hold the softmax denominator (64x replicated)
    v_sb = [persist.tile([128, NHEAD, 2 * DH], BF16, tag=f"v{i}", name=f"v{i}")
            for i in range(8)]
    vout = [[persist.tile([128, 1024], BF16, tag=f"vout{i}_{h}", name=f"vout{i}_{h}")
             for h in range(2)] for i in range(4)]
    # per-half tiles (separate tiles so half-0 readers don't pick up false
    # dependencies on half-1 writers through whole-tile tracking)
    x_raw = [[persist.tile([128, 512], F32R, tag=f"xraw{h}_{i}",
                           name=f"xraw{h}_{i}") for i in range(4)]
             for h in range(2)]
    xs_ln = [[persist.tile([128, 512], BF16, tag=f"xsln{h}_{i}",
                           name=f"xsln{h}_{i}") for i in range(4)]
             for h in range(2)]
    xs_ext2 = [small.tile([2, 512], BF16, name=f"xs_ext2_{h}")
               for h in range(2)]          # row0 = -mu*rstd, row1 = ones (DMA)
    rstd_bc = [small.tile([128, 512], F32, name=f"rstd_bc_{h}")
               for h in range(2)]

    bsr_sb = small.tile([128, 4], F32)
    bp_sb = small.tile([128, 4], F32)
    ones_c = small.tile([128, 1], F32R)
    # LN row tensors ([1, N] tiles, base partition 0)
    sm_row = [small.tile([1, 512], F32, name=f"sm_row{h}") for h in range(2)]
    mq_row = [small.tile([1, 512], F32, name=f"mq_row{h}") for h in range(2)]
    vr_row = [small.tile([1, 512], F32, name=f"vr_row{h}") for h in range(2)]
    rstd_row = [small.tile([1, 512], F32, name=f"rstd_row{h}") for h in range(2)]
    warm = small.tile([1, 8], F32)
    warm2 = small.tile([1, 8], F32)
    warm3 = small.tile([1, 8], F32)

    # ---------------- DMA in (interleaved so conv can start early) -------
    nc.sync.dma_start(out=xh0[:, 0, :], in_=xq[:, 0, :])
    for didj in range(4):
        nc.sync.dma_start(out=w2_sb[0][:, didj, :], in_=w2[0, :, didj, :])
    nc.sync.dma_start(out=xh0[:, 1, :], in_=xq[:, 1, :])
    nc.sync.dma_start(out=w2_sb[1][:], in_=w2[1])
    # warm the ACT exp table set under the DMA head
    nc.vector.memset(warm[:], 1.0)
    nc.scalar.activation(warm2[:], warm[:], AF.Exp)
    for ct in range(2, 4):
        nc.sync.dma_start(out=xh0[:, ct, :], in_=xq[:, ct, :])
        nc.sync.dma_start(out=w2_sb[ct][:], in_=w2[ct])
    nc.sync.dma_start(out=bsr_sb[:], in_=bsr_t)
    for ct in range(4):
        nc.sync.dma_start(out=xh1[:, ct, :], in_=xo[:, ct, :])
    nc.sync.dma_start(out=wkg_sb[:], in_=wkg)
    nc.sync.dma_start(out=wkg2_sb[:], in_=wkg2)
    nc.sync.dma_start(out=wvg_sb[:], in_=wvg)
    nc.sync.dma_start(out=wvg2_sb[:], in_=wvg2)
    nc.sync.dma_start(out=wq_sb[:], in_=wq)
    nc.sync.dma_start(out=wp_sb[:], in_=wp)
    nc.sync.dma_start(out=bp_sb[:], in_=bp_t)
    nc.sync.dma_start(out=ones_c[:], in_=ones_c1)
    nc.sync.dma_start(out=xs_ext2[0][1:2, :], in_=ones_row[0:1, 0:512])
    nc.sync.dma_start(out=xs_ext2[1][1:2, :], in_=ones_row[0:1, 0:512])
    # ones-columns FIRST: av psum rows 0..63 = softmax denominator (base
    # partition 0, so the reciprocal custom-op operands stay base-aligned)
    for kt in range(8):
        nc.vector.memset(v_sb[kt][:, :, 0:DH], 1.0)
    wz = small.tile([128, 512], BF16)
    with tc.high_priority():
        nc.vector.memset(wz[:], 0.0)

    inv_c = 1.0 / C

    # ================= Phase A: conv -> LN -> KV -> q(chunk0) ============
    with tc.tile_pool(name="ppa", bufs=4, space="PSUM") as ppa, \
         tc.tile_pool(name="px", bufs=2, space="PSUM") as px, \
         tc.tile_pool(name="pxsq", bufs=4) as pxsq:

        def conv_half(half, xh):
            hsl = slice(half * 512, (half + 1) * 512)
            # one psum tile per output-channel block: write-after-read deps
            # stay per-tile, so ot+1 matmuls don't wait on ot's evac
            pss = [ppa.tile([128, 512], F32, tag="conv", name=f"conv{half}_{ot}")
                   for ot in range(4)]
            psv = [pss[ot][:].rearrange("p (a b) -> p a b", a=16)
                   for ot in range(4)]

            def mm(ct, didj, ot, start, stop):
                di, dj = didj // 2, didj % 2
                rhs = bass.AP(
                    tensor=xh[:].tensor,
                    offset=xh[:].offset + ct * NQ + di * WS + dj,
                    ap=[xh[:].ap[0], [2 * WS, 16], [2, 32]],
                )
                nc.tensor.matmul(
                    psv[ot], lhsT=w2_sb[ct][:, didj, ot * 128:(ot + 1) * 128],
                    rhs=rhs, start=start, stop=stop)

            for ct in range(3):
                for didj in range(4):
                    for ot in range(4):
                        mm(ct, didj, ot, start=(ct == 0 and didj == 0), stop=False)
            xsq = []
            for ot in range(4):   # last ct pass ot-major so evacs pipeline
                for didj in range(4):
                    mm(3, didj, ot, start=False, stop=(didj == 3))
                nc.scalar.activation(x_raw[half][ot][:], pss[ot][:], AF.Identity,
                                     bias=bsr_sb[:, ot:ot + 1])
                t = pxsq.tile([128, 512], F32R, tag="xsq", name="xsq")
                nc.vector.tensor_mul(t[:], x_raw[half][ot][:].bitcast(F32),
                                     x_raw[half][ot][:].bitcast(F32))
                xsq.append(t)
            return xsq

        def stats_half(half, xsq):
            ps = px.tile([128, 1024], F32, tag="st", name=f"st{half}")
            with tc.high_priority():   # schedule as soon as x_raw/xsq land,
                for ct in range(4):    # not after the other half's conv
                    nc.tensor.matmul(ps[0:1, 0:512], lhsT=ones_c[:],
                                     rhs=x_raw[half][ct][:],
                                     start=(ct == 0), stop=(ct == 3))
                for ct in range(4):
                    nc.tensor.matmul(ps[0:1, 512:1024], lhsT=ones_c[:],
                                     rhs=xsq[ct][:],
                                     start=(ct == 0), stop=(ct == 3))
            return ps

        def ln_half_ops(half, ps):
            """LN rstd chain as a list of thunks so the emitter can
            interleave them between kv psum-groups (keeps the DVE FIFO from
            head-of-line blocking on the serial chain)."""
            sm, mq = sm_row[half][0:1, :], mq_row[half][0:1, :]
            vr, rs = vr_row[half][0:1, :], rstd_row[half][0:1, :]
            ops = []
            ops.append(lambda: nc.vector.tensor_scalar_mul(
                sm, ps[0:1, 0:512], inv_c))
            ops.append(lambda: nc.vector.tensor_mul(mq, sm, sm))
            ops.append(lambda: nc.vector.scalar_tensor_tensor(
                vr, ps[0:1, 512:1024], inv_c, mq,
                op0=ALU.mult, op1=ALU.subtract))
            ops.append(lambda: nc.vector.tensor_scalar_add(vr, vr, EPS))
            ops.append(lambda: nc.vector.reciprocal_approx_fast(
                out=rs, in_=vr))
            for _ in range(3):
                ops.append(lambda: nc.vector.tensor_mul(mq, rs, rs))
                ops.append(lambda: nc.vector.scalar_tensor_tensor(
                    mq, vr, -0.5, mq, op0=ALU.mult, op1=ALU.mult))
                ops.append(lambda: nc.vector.scalar_tensor_tensor(
                    rs, mq, 1.5, rs, op0=ALU.add, op1=ALU.mult))
            ops.append(lambda: nc.gpsimd.partition_broadcast(
                rstd_bc[half][:], rs))
            for ct in range(4):
                ops.append(lambda ct=ct: nc.vector.tensor_mul(
                    xs_ln[half][ct][:],
                    x_raw[half][ct][:].bitcast(F32), rstd_bc[half][:]))
            ops.append(lambda: nc.vector.scalar_tensor_tensor(
                xs_ext2[half][0:1, :], sm, -1.0, rs,
                op0=ALU.mult, op1=ALU.mult))
            return ops

        def drain(ops, n):
            for _ in range(min(n, len(ops))):
                ops.pop(0)()

        def kv_half(half, interleave=None):
            # psum evacs go to the (phase-A idle) scalar engine so the DVE
            # FIFO stays free for the interleaved LN chain
            kTh = kT0 if half == 0 else kT1
            for ot in range(4):
                ps = px.tile([128, 1024], F32, tag="st", name="ps_k")
                for ct in range(4):
                    nc.tensor.matmul(ps[:, 0:512],
                                     lhsT=wkg_sb[:, ct, ot * 128:(ot + 1) * 128],
                                     rhs=xs_ln[half][ct][:],
                                     start=(ct == 0), stop=False)
                nc.tensor.matmul(ps[:, 0:512],
                                 lhsT=wkg2_sb[:, ot * 128:(ot + 1) * 128],
                                 rhs=xs_ext2[half][:], start=False, stop=True)
                nc.scalar.copy(kTh[ot][:], ps[:, 0:512])
                if interleave:
                    drain(interleave, 2)
            for tt in range(half * 4, half * 4 + 4):
                sl = slice((tt % 4) * 128, (tt % 4) * 128 + 128)
                ps = px.tile([128, 1024], F32, tag="st", name="ps_v")
                for ct in range(4):
                    nc.tensor.matmul(ps[:, 0:512], lhsT=xs_ln[half][ct][:, sl],
                                     rhs=wvg_sb[:, ct, :],
                                     start=(ct == 0), stop=False)
                nc.tensor.matmul(ps[:, 0:512], lhsT=xs_ext2[half][:, sl],
                                 rhs=wvg2_sb[:], start=False, stop=True)
                nc.scalar.copy(
                    v_sb[tt][:, :, DH:2 * DH],
                    ps[:, 0:512].rearrange("p (h d) -> p h d", h=NHEAD))
                if interleave:
                    drain(interleave, 3)

        def q_chunk_px(ot, qc):
            ps = px.tile([128, 1024], F32, tag="st", name="ps_q")
            for ct in range(4):
                nc.tensor.matmul(
                    ps[:, 0:512],
                    lhsT=wq_sb[:, ct, ot * 128:(ot + 1) * 128],
                    rhs=xh0[:, ct, qc * 512:(qc + 1) * 512],
                    start=(ct == 0), stop=(ct == 3))
            nc.scalar.copy(qT[ot][:, qc * 512:(qc + 1) * 512],
                           ps[:, 0:512])

        with tc.high_priority():    # ~3.5us of dummy matmuls under the DMA
            pw = px.tile([128, 1024], F32, tag="st", name="pe_warm")
            for i in range(16):
                nc.tensor.matmul(pw[:, 0:512], lhsT=wz[0:128, 0:128],
                                 rhs=wz[:], start=(i == 0), stop=(i == 15))
        xsq0 = conv_half(0, xh0)
        st0 = stats_half(0, xsq0)
        ops0 = ln_half_ops(0, st0)
        drain(ops0, 5)             # stats0-dependent head runs under conv1
        xsq1 = conv_half(1, xh1)
        drain(ops0, 99)
        st1 = stats_half(1, xsq1)
        ops1 = ln_half_ops(1, st1)
        for qc in range(2):        # q chunks are dep-free: Tile slots them
            for ot in range(4):    # into the PE idle under the LN chains
                q_chunk_px(ot, qc)
        kv_half(0, interleave=ops1)   # LN1 chain interleaves with kv0 evacs
        drain(ops1, 99)
        kv_half(1)

    # ================= Phase B: attention ================================
    with tc.tile_pool(name="pp", bufs=2, space="PSUM") as pp, \
         tc.tile_pool(name="pav", bufs=4, space="PSUM") as pav, \
         tc.tile_pool(name="pexp", bufs=6) as pexp, \
         tc.tile_pool(name="prb", bufs=2) as prb, \
         tc.tile_pool(name="pyb", bufs=2) as pyb:

        def q_chunk(ot, qc):
            ps = pav.tile([128, 512], F32, tag="av", name="ps_qd")
            for ct in range(4):
                nc.tensor.matmul(
                    ps[:], lhsT=wq_sb[:, ct, ot * 128:(ot + 1) * 128],
                    rhs=xh0[:, ct, qc * 512:(qc + 1) * 512],
                    start=(ct == 0), stop=(ct == 3))
            nc.vector.tensor_copy(qT[ot][:, qc * 512:(qc + 1) * 512], ps[:])

        def proj_chunk(qc, ot, evac_scalar=False):
            qh, qr = qc // 2, (qc % 2) * 512
            ps = pav.tile([128, 512], F32, tag="av", name="ps_proj")
            for ct in range(4):
                nc.tensor.matmul(
                    ps[:], lhsT=wp_sb[:, ct, ot * 128:(ot + 1) * 128],
                    rhs=vout[ct][qh][:, qr:qr + 512],
                    start=(ct == 0), stop=(ct == 3))
            yb = pyb.tile([128, 512], BF16, tag="yb", name="yb")
            if evac_scalar:   # tail: ACT is idle after the last exp
                nc.scalar.activation(yb[:], ps[:], AF.Identity,
                                     bias=bp_sb[:, ot:ot + 1])
            else:
                nc.vector.tensor_scalar_add(yb[:], ps[:], bp_sb[:, ot:ot + 1])
            nc.sync.dma_start(
                out=yt[ot * 128:(ot + 1) * 128, qc * 512:(qc + 1) * 512],
                in_=yb[:])

        def score_pair(hp, qc, kt):
            kTh = kT0 if kt < 4 else kT1
            ksl = slice((kt % 4) * 128, (kt % 4) * 128 + 128)
            qsl = slice(qc * 512, (qc + 1) * 512)
            sc = pp.tile([128, 1024], F32, tag="sc", name="sc")
            for sub in range(2):
                rr = sub * 64
                nc.tensor.matmul(
                    sc[:, sub * 512:(sub + 1) * 512],
                    lhsT=kTh[hp][rr:rr + 64, ksl],
                    rhs=qT[hp][rr:rr + 64, qsl],
                    start=True, stop=True)
            ex = pexp.tile([128, 1024], BF16, tag="ex", name="ex")
            nc.scalar.activation(ex[:], sc[:], AF.Exp, scale=SCALE)
            return ex

        # global software pipeline: the score/exp stream leads the av
        # stream by one kt slot and runs continuously ACROSS unit
        # boundaries, so the exp engine never waits at a boundary
        units = [(hp, qc) for qc in range(4) for hp in range(4)]
        blocks = [(i, 0, 8) for i in range(16)]
        seq = [(i, kt) for (i, b, e) in blocks for kt in range(b, e)]
        av_tiles = {}
        ex_p = score_pair(units[seq[0][0]][0], units[seq[0][0]][1], seq[0][1])
        for n, (idx, kt) in enumerate(seq):
            hp, qc = units[idx]
            qh, qr = qc // 2, (qc % 2) * 512
            if kt == 0:
                av_tiles[idx] = (
                    pav.tile([128, 512], F32, tag="av", name="av0"),
                    pav.tile([128, 512], F32, tag="av", name="av1"))
            av0, av1 = av_tiles[idx]
            ex = ex_p
            if n + 1 < len(seq):
                ni, nkt = seq[n + 1]
                ex_p = score_pair(units[ni][0], units[ni][1], nkt)
            nc.tensor.matmul(av0[:], lhsT=v_sb[kt][:, 2 * hp, :],
                             rhs=ex[:, 0:512],
                             start=(kt == 0), stop=(kt == 7))
            nc.tensor.matmul(av1[:], lhsT=v_sb[kt][:, 2 * hp + 1, :],
                             rhs=ex[:, 512:1024],
                             start=(kt == 0), stop=(kt == 7))
            if kt == 2 and qc in (1, 2):
                tc.cur_priority += 14   # filler work: schedule after the
                q_chunk(hp, qc + 1)     # score/exp stream
                tc.cur_priority -= 14
            if kt == 5 and qc > 0:
                tc.cur_priority += 14
                proj_chunk(qc - 1, hp)
                tc.cur_priority -= 14
            if kt == 7:
                for h, av in ((0, av0), (1, av1)):
                    rbc = prb.tile([64, 512], F32, tag="rbc", name="rbc")
                    nc.vector.reciprocal_approx_fast(out=rbc[:],
                                                     in_=av[0:64, :])
                    nc.vector.tensor_mul(
                        vout[hp][qh][h * 64:(h + 1) * 64, qr:qr + 512],
                        av[64:128, :], rbc[:])
        for ot in range(4):
            proj_chunk(3, ot, evac_scalar=True)


_CACHE = {}


def _build():
    if "nc" in _CACHE:
        return _CACHE["nc"]
    nc = bacc.Bacc("TRN2", target_bir_lowering=False, debug=False, num_devices=8)
    io = (
        nc.dram_tensor("xq", [128, 4, NQ], BF16, kind="ExternalInput").ap(),
        nc.dram_tensor("xo", [128, 4, NQ], BF16, kind="ExternalInput").ap(),
        nc.dram_tensor("w2", [4, 128, 4, C], BF16, kind="ExternalInput").ap(),
        nc.dram_tensor("wq", [128, 4, C], BF16, kind="ExternalInput").ap(),
        nc.dram_tensor("wkg", [128, 4, C], BF16, kind="ExternalInput").ap(),
        nc.dram_tensor("wkg2", [2, C], BF16, kind="ExternalInput").ap(),
        nc.dram_tensor("wvg", [128, 4, C], BF16, kind="ExternalInput").ap(),
        nc.dram_tensor("wvg2", [2, C], BF16, kind="ExternalInput").ap(),
        nc.dram_tensor("wp", [128, 4, C], BF16, kind="ExternalInput").ap(),
        nc.dram_tensor("bsr_t", [128, 4], F32, kind="ExternalInput").ap(),
        nc.dram_tensor("bp_t", [128, 4], F32, kind="ExternalInput").ap(),
        nc.dram_tensor("ones_row", [1, NKV], BF16, kind="ExternalInput").ap(),
        nc.dram_tensor("ones_c1", [128, 1], F32R, kind="ExternalInput").ap(),
        nc.dram_tensor("yt", [C, NQ], BF16, kind="ExternalOutput").ap(),
    )
    with tile.TileContext(nc) as tc, ExitStack() as ctx:
        _emit(nc, tc, ctx, io)
    nc.compile()
    _CACHE["nc"] = nc
    return nc


def _prep_inputs(x, Wq, Wkv, Wsr, bsr, ln_g, ln_b, Wp, bp):
    x = np.asarray(x, np.float32)
    Wq = np.asarray(Wq, np.float32)
    Wkv = np.asarray(Wkv, np.float32)
    Wsr = np.asarray(Wsr, np.float32)
    bsr = np.asarray(bsr, np.float32)
    ln_g = np.asarray(ln_g, np.float32)
    ln_b = np.asarray(ln_b, np.float32)
    Wp = np.asarray(Wp, np.float32)
    bp = np.asarray(bp, np.float32)

    def pct(a):
        return np.ascontiguousarray(a.reshape(4, 128, -1).transpose(1, 0, 2))

    # w2[ct][128(part=in-ch), didj, out-ch]
    w2 = np.ascontiguousarray(
        Wsr.transpose(2, 3, 1, 0).reshape(4, 4, 128, C).transpose(1, 2, 0, 3)
        .astype(BF))
    wq = pct(Wq.T.astype(BF))
    Wk, Wv = Wkv[:C], Wkv[C:]

    def ext(W):
        main = pct((W * ln_g[None, :]).T.astype(BF))                    # [p, ct, o]
        rows = np.stack([W @ ln_g, W @ ln_b]).astype(BF)                # [2, o]
        return main, np.ascontiguousarray(rows)

    wkg, wkg2 = ext(Wk)
    wvg, wvg2 = ext(Wv)
    wp = pct(Wp.T.astype(BF))
    bsr_t = np.ascontiguousarray(bsr.reshape(4, 128).T)
    bp_t = np.ascontiguousarray(bp.reshape(4, 128).T)

    shared = dict(w2=w2, wq=wq, wkg=wkg, wkg2=wkg2, wvg=wvg, wvg2=wvg2,
                  wp=wp, bsr_t=bsr_t, bp_t=bp_t,
                  ones_row=np.ones((1, NKV), BF),
                  ones_c1=np.ones((128, 1), np.float32))
    in_maps = []
    for core in range(8):
        b, half = core // 2, core % 2
        xT = x[b].T.astype(BF)                # [C, NTOK]
        m = dict(shared)
        m["xq"] = pct(xT[:, half * NQ:(half + 1) * NQ])
        m["xo"] = pct(xT[:, (1 - half) * NQ:(2 - half) * NQ])
        in_maps.append(m)
    return in_maps


def kernel(x, H, W, Wq, Wkv, Wsr, bsr, ln_g, ln_b, Wp, bp, _trace=False):
    nc = _build()
    in_maps = _prep_inputs(x, Wq, Wkv, Wsr, bsr, ln_g, ln_b, Wp, bp)
    res = run_bass_kernel_spmd(nc, in_maps, list(range(8)), trace=_trace)
    y = np.empty((B, NTOK, C), np.float32)
    for core in range(8):
        b, half = core // 2, core % 2
        y[b, half * NQ:(half + 1) * NQ, :] = \
            res.results[core]["yt"].astype(np.float32).T
    kernel._last_result = res
    return y


# revision 62
# speedup vs baseline: 1.1930x; 1.0109x over previous
"""Trainium2 Bass kernel for AttentionSR (spatial-reduction attention), v2.1.

Reference computation (per batch b):
  q = x @ Wq.T                                   [4096, 512] -> heads [8, 4096, 64]
  x_ = conv2x2_stride2(x as NCHW image, Wsr) + bsr   -> [1024, 512]
  x_ = layernorm(x_, g, b)
  k, v = split(x_ @ Wkv.T)                       [8, 1024, 64] each
  out = softmax(q k^T / 8) v                     -> [4096, 512]
  y = out @ Wp.T + bp

Sharding (8 cores): core = 2*batch + query_half. Each core owns one batch's
conv/LN/KV (duplicated across the pair) and 2048 of its 4096 query rows.
No collectives.

v2 design notes (attention phase is ACT(exp)-throughput-bound: 128 exp
instructions of [128,1024] ~= 136us; everything else hides under it):
  - channel-major layout throughout, no PE transposes.
  - conv is ct(outer)-ordered so it can start as soon as the first input
    channel-block DMA lands; x DMA is split per channel-block.
  - v2.1: LN rstd = Exp(-0.5*Ln(var+eps)) on the scalar engine (same
    natural-log/exp table set as the attention exp -> no table switch),
    replacing the DVE Newton chain whose ~570ns/hop semaphore latency
    made it ~10us of serial critical path.
  - phase A PE stream has no bubbles: conv0, stats0, conv1 (LN0 on ACT/DVE
    underneath), stats1, kv0 (LN1 underneath), kv1, q(first 512 cols).
  - attention: per (head-pair, q-512-chunk): 8x { score pair (row-packed
    64-row matmuls), exp, one av matmul per head }.  av stationary is
    [ones64 | v] -> M=128, so psum rows 0..63 hold the softmax
    denominator replicated 64x: evac is reciprocal[64,512] + multiply,
    no partition-broadcast, no 1-lane row ops.
  - av psums are [128,512] (1 bank); scores 2x[128,1024]; 1 spare bank
    cycles between deferred q-projection chunks (qc+1) and output
    projection chunks (qc-1), interleaved into the attention PE slack.
  - output y is written bf16.
"""

import numpy as np
import ml_dtypes
from contextlib import ExitStack

import concourse.bass as bass
import concourse.bacc as bacc
import concourse.tile as tile
from concourse import mybir
from concourse.bass_utils import run_bass_kernel_spmd

BF = ml_dtypes.bfloat16
F32 = mybir.dt.float32
F32R = mybir.dt.float32r
BF16 = mybir.dt.bfloat16
AF = mybir.ActivationFunctionType
ALU = mybir.AluOpType

C = 512          # model dim
NHEAD = 8
DH = 64          # head dim
HS = WS = 64     # image height/width
NTOK = HS * WS   # 4096 tokens per batch
NQ = 2048        # query rows per core
NKV = 1024       # reduced tokens (keys)
B = 4
SCALE = DH ** -0.5
EPS = 1e-5


def _emit(nc, tc, ctx, io):
    (xq, xo, w2, wq, wkg, wkg2, wvg, wvg2, wp, bsr_t, bp_t,
     ones_row, ones_c1, yt) = io

    persist = ctx.enter_context(tc.tile_pool(name="persist", bufs=1))
    small = ctx.enter_context(tc.tile_pool(name="small", bufs=1))

    # ---- persistent sbuf tensors ----
    xh0 = persist.tile([128, 4, NQ], BF16, tag="xh0")
    xh1 = persist.tile([128, 4, NQ], BF16, tag="xh1")
    w2_sb = [persist.tile([128, 4, C], BF16, tag=f"w2_{i}", name=f"w2_{i}") for i in range(4)]
    wq_sb = persist.tile([128, 4, C], BF16, tag="wq")
    wkg_sb = persist.tile([128, 4, C], BF16, tag="wkg")
    wkg2_sb = persist.tile([2, C], BF16, tag="wkg2")
    wvg_sb = persist.tile([128, 4, C], BF16, tag="wvg")
    wvg2_sb = persist.tile([2, C], BF16, tag="wvg2")
    wp_sb = persist.tile([128, 4, C], BF16, tag="wp")

    qT = [persist.tile([128, NQ], BF16, tag=f"qT{i}", name=f"qT{i}") for i in range(4)]
    kT0 = [persist.tile([128, 512], BF16, tag=f"kT0{i}", name=f"kT0{i}") for i in range(4)]
    kT1 = [persist.tile([128, 512], BF16, tag=f"kT1{i}", name=f"kT1{i}") for i in range(4)]
    # v with 64 ones-columns appended per head: av matmul M=128, psum rows
    # 64..127 hold the softmax denominator (64x replicated)
    v_sb = [persist.tile([128, NHEAD, 2 * DH], BF16, tag=f"v{i}", name=f"v{i}")
            for i in range(8)]
    vout = [[persist.tile([128, 1024], BF16, tag=f"vout{i}_{h}", name=f"vout{i}_{h}")
             for h in range(2)] for i in range(4)]
    # per-half tiles (separate tiles so half-0 readers don't pick up false
    # dependencies on half-1 writers through whole-tile tracking)
    x_raw = [[persist.tile([128, 512], F32R, tag=f"xraw{h}_{i}",
                           name=f"xraw{h}_{i}") for i in range(4)]
             for h in range(2)]
    xs_ln = [[persist.tile([128, 512], BF16, tag=f"xsln{h}_{i}",
                           name=f"xsln{h}_{i}") for i in range(4)]
             for h in range(2)]
    xs_ext2 = [small.tile([2, 512], BF16, name=f"xs_ext2_{h}")
               for h in range(2)]          # row0 = -mu*rstd, row1 = ones (DMA)
    rstd_bc = [small.tile([128, 512], F32, name=f"rstd_bc_{h}")
               for h in range(2)]

    bsr_sb = small.tile([128, 4], F32)
    bp_sb = small.tile([128, 4], F32)
    ones_c = small.tile([128, 1], F32R)
    # LN row tensors ([1, N] tiles, base partition 0)
    sm_row = [small.tile([1, 512], F32, name=f"sm_row{h}") for h in range(2)]
    mq_row = [small.tile([1, 512], F32, name=f"mq_row{h}") for h in range(2)]
    vr_row = [small.tile([1, 512], F32, name=f"vr_row{h}") for h in range(2)]
    rstd_row = [small.tile([1, 512], F32, name=f"rstd_row{h}") for h in range(2)]
    warm = small.tile([1, 8], F32)
    warm2 = small.tile([1, 8], F32)
    warm3 = small.tile([1, 8], F32)

    # ---------------- DMA in (interleaved so conv can start early) -------
    nc.sync.dma_start(out=xh0[:, 0, :], in_=xq[:, 0, :])
    for didj in range(4):
        nc.sync.dma_start(out=w2_sb[0][:, didj, :], in_=w2[0, :, didj, :])
    nc.sync.dma_start(out=xh0[:, 1, :], in_=xq[:, 1, :])
    nc.sync.dma_start(out=w2_sb[1][:], in_=w2[1])
    # warm the ACT exp table set under the DMA head
    nc.vector.memset(warm[:], 1.0)
    nc.scalar.activation(warm2[:], warm[:], AF.Exp)
    for ct in range(2, 4):
        nc.sync.dma_start(out=xh0[:, ct, :], in_=xq[:, ct, :])
        nc.sync.dma_start(out=w2_sb[ct][:], in_=w2[ct])
    nc.sync.dma_start(out=bsr_sb[:], in_=bsr_t)
    for ct in range(4):
        nc.sync.dma_start(out=xh1[:, ct, :], in_=xo[:, ct, :])
    nc.sync.dma_start(out=wkg_sb[:], in_=wkg)
    nc.sync.dma_start(out=wkg2_sb[:], in_=wkg2)
    nc.sync.dma_start(out=wvg_sb[:], in_=wvg)
    nc.sync.dma_start(out=wvg2_sb[:], in_=wvg2)
    nc.sync.dma_start(out=wq_sb[:], in_=wq)
    nc.sync.dma_start(out=wp_sb[:], in_=wp)
    nc.sync.dma_start(out=bp_sb[:], in_=bp_t)
    nc.sync.dma_start(out=ones_c[:], in_=ones_c1)
    nc.sync.dma_start(out=xs_ext2[0][1:2, :], in_=ones_row[0:1, 0:512])
    nc.sync.dma_start(out=xs_ext2[1][1:2, :], in_=ones_row[0:1, 0:512])
    # ones-columns FIRST: av psum rows 0..63 = softmax denominator (base
    # partition 0, so the reciprocal custom-op operands stay base-aligned)
    for kt in range(8):
        nc.vector.memset(v_sb[kt][:, :, 0:DH], 1.0)
    wz = small.tile([128, 512], BF16)
    with tc.high_priority():
        nc.vector.memset(wz[:], 0.0)

    inv_c = 1.0 / C

    # ================= Phase A: conv -> LN -> KV -> q(chunk0) ============
    with tc.tile_pool(name="ppa", bufs=4, space="PSUM") as ppa, \
         tc.tile_pool(name="px", bufs=2, space="PSUM") as px, \
         tc.tile_pool(name="pxsq", bufs=4) as pxsq:

        def conv_half(half, xh):
            hsl = slice(half * 512, (half + 1) * 512)
            # one psum tile per output-channel block: write-after-read deps
            # stay per-tile, so ot+1 matmuls don't wait on ot's evac
            pss = [ppa.tile([128, 512], F32, tag="conv", name=f"conv{half}_{ot}")
                   for ot in range(4)]
            psv = [pss[ot][:].rearrange("p (a b) -> p a b", a=16)
                   for ot in range(4)]

            def mm(ct, didj, ot, start, stop):
                di, dj = didj // 2, didj % 2
                rhs = bass.AP(
                    tensor=xh[:].tensor,
                    offset=xh[:].offset + ct * NQ + di * WS + dj,
                    ap=[xh[:].ap[0], [2 * WS, 16], [2, 32]],
                )
                nc.tensor.matmul(
                    psv[ot], lhsT=w2_sb[ct][:, didj, ot * 128:(ot + 1) * 128],
                    rhs=rhs, start=start, stop=stop)

            for ct in range(3):
                for didj in range(4):
                    for ot in range(4):
                        mm(ct, didj, ot, start=(ct == 0 and didj == 0), stop=False)
            xsq = []
            for ot in range(4):   # last ct pass ot-major so evacs pipeline
                for didj in range(4):
                    mm(3, didj, ot, start=False, stop=(didj == 3))
                nc.scalar.activation(x_raw[half][ot][:], pss[ot][:], AF.Identity,
                                     bias=bsr_sb[:, ot:ot + 1])
                t = pxsq.tile([128, 512], F32R, tag="xsq", name="xsq")
                nc.vector.tensor_mul(t[:], x_raw[half][ot][:].bitcast(F32),
                                     x_raw[half][ot][:].bitcast(F32))
                xsq.append(t)
            return xsq

        def stats_half(half, xsq):
            ps = px.tile([128, 1024], F32, tag="st", name=f"st{half}")
            with tc.high_priority():   # schedule as soon as x_raw/xsq land,
                for ct in range(4):    # not after the other half's conv
                    nc.tensor.matmul(ps[0:1, 0:512], lhsT=ones_c[:],
                                     rhs=x_raw[half][ct][:],
                                     start=(ct == 0), stop=(ct == 3))
                for ct in range(4):
                    nc.tensor.matmul(ps[0:1, 512:1024], lhsT=ones_c[:],
                                     rhs=xsq[ct][:],
                                     start=(ct == 0), stop=(ct == 3))
            return ps

        def ln_half_ops(half, ps):
            """LN rstd chain as a list of thunks so the emitter can
            interleave them between kv psum-groups (keeps the DVE FIFO from
            head-of-line blocking on the serial chain)."""
            sm, mq = sm_row[half][0:1, :], mq_row[half][0:1, :]
            vr, rs = vr_row[half][0:1, :], rstd_row[half][0:1, :]
            ops = []
            ops.append(lambda: nc.vector.tensor_scalar_mul(
                sm, ps[0:1, 0:512], inv_c))
            ops.append(lambda: nc.vector.tensor_mul(mq, sm, sm))
            ops.append(lambda: nc.vector.scalar_tensor_tensor(
                vr, ps[0:1, 512:1024], inv_c, mq,
                op0=ALU.mult, op1=ALU.subtract))
            ops.append(lambda: nc.vector.tensor_scalar_add(vr, vr, EPS))
            ops.append(lambda: nc.vector.reciprocal_approx_fast(
                out=rs, in_=vr))
            for _ in range(2):   # 2 Newton iters: rstd err ~2e-4, and each
                # chain hop costs ~1.25us of phase-A critical path
                ops.append(lambda: nc.vector.tensor_mul(mq, rs, rs))
                ops.append(lambda: nc.vector.scalar_tensor_tensor(
                    mq, vr, -0.5, mq, op0=ALU.mult, op1=ALU.mult))
                ops.append(lambda: nc.vector.scalar_tensor_tensor(
                    rs, mq, 1.5, rs, op0=ALU.add, op1=ALU.mult))
            ops.append(lambda: nc.gpsimd.partition_broadcast(
                rstd_bc[half][:], rs))
            for ct in range(4):
                ops.append(lambda ct=ct: nc.vector.tensor_mul(
                    xs_ln[half][ct][:],
                    x_raw[half][ct][:].bitcast(F32), rstd_bc[half][:]))
            ops.append(lambda: nc.vector.scalar_tensor_tensor(
                xs_ext2[half][0:1, :], sm, -1.0, rs,
                op0=ALU.mult, op1=ALU.mult))
            return ops

        def drain(ops, n):
            for _ in range(min(n, len(ops))):
                ops.pop(0)()

        def kv_half(half, interleave=None):
            # psum evacs go to the (phase-A idle) scalar engine so the DVE
            # FIFO stays free for the interleaved LN chain
            kTh = kT0 if half == 0 else kT1
            for ot in range(4):
                ps = px.tile([128, 1024], F32, tag="st", name="ps_k")
                for ct in range(4):
                    nc.tensor.matmul(ps[:, 0:512],
                                     lhsT=wkg_sb[:, ct, ot * 128:(ot + 1) * 128],
                                     rhs=xs_ln[half][ct][:],
                                     start=(ct == 0), stop=False)
                nc.tensor.matmul(ps[:, 0:512],
                                 lhsT=wkg2_sb[:, ot * 128:(ot + 1) * 128],
                                 rhs=xs_ext2[half][:], start=False, stop=True)
                nc.scalar.copy(kTh[ot][:], ps[:, 0:512])
                if interleave:
                    drain(interleave, 2)
            for tt in range(half * 4, half * 4 + 4):
                sl = slice((tt % 4) * 128, (tt % 4) * 128 + 128)
                ps = px.tile([128, 1024], F32, tag="st", name="ps_v")
                for ct in range(4):
                    nc.tensor.matmul(ps[:, 0:512], lhsT=xs_ln[half][ct][:, sl],
                                     rhs=wvg_sb[:, ct, :],
                                     start=(ct == 0), stop=False)
                nc.tensor.matmul(ps[:, 0:512], lhsT=xs_ext2[half][:, sl],
                                 rhs=wvg2_sb[:], start=False, stop=True)
                nc.scalar.copy(
                    v_sb[tt][:, :, DH:2 * DH],
                    ps[:, 0:512].rearrange("p (h d) -> p h d", h=NHEAD))
                if interleave:
                    drain(interleave, 3)

        def q_chunk_px(ot, qc):
            ps = px.tile([128, 1024], F32, tag="st", name="ps_q")
            for ct in range(4):
                nc.tensor.matmul(
                    ps[:, 0:512],
                    lhsT=wq_sb[:, ct, ot * 128:(ot + 1) * 128],
                    rhs=xh0[:, ct, qc * 512:(qc + 1) * 512],
                    start=(ct == 0), stop=(ct == 3))
            nc.scalar.copy(qT[ot][:, qc * 512:(qc + 1) * 512],
                           ps[:, 0:512])

        with tc.high_priority():    # ~3.4us of dummy matmuls under the DMA
            pw = px.tile([128, 1024], F32, tag="st", name="pe_warm")
            for i in range(12):     # just enough to cover the HAM window
                nc.tensor.matmul(pw[:, 0:512], lhsT=wz[0:128, 0:128],
                                 rhs=wz[:], start=(i == 0), stop=(i == 11))
        xsq0 = conv_half(0, xh0)
        st0 = stats_half(0, xsq0)
        ops0 = ln_half_ops(0, st0)
        drain(ops0, 5)             # stats0-dependent head runs under conv1
        xsq1 = conv_half(1, xh1)
        drain(ops0, 99)
        st1 = stats_half(1, xsq1)
        ops1 = ln_half_ops(1, st1)
        for qc in range(2):        # q chunks are dep-free: Tile slots them
            for ot in range(4):    # into the PE idle under the LN chains
                q_chunk_px(ot, qc)
        kv_half(0, interleave=ops1)   # LN1 chain interleaves with kv0 evacs
        drain(ops1, 99)
        kv_half(1)

    # ================= Phase B: attention ================================
    with tc.tile_pool(name="pp", bufs=2, space="PSUM") as pp, \
         tc.tile_pool(name="pav", bufs=4, space="PSUM") as pav, \
         tc.tile_pool(name="pexp", bufs=6) as pexp, \
         tc.tile_pool(name="prb", bufs=2) as prb, \
         tc.tile_pool(name="pyb", bufs=2) as pyb:

        def q_chunk(ot, qc):
            ps = pav.tile([128, 512], F32, tag="av", name="ps_qd")
            for ct in range(4):
                nc.tensor.matmul(
                    ps[:], lhsT=wq_sb[:, ct, ot * 128:(ot + 1) * 128],
                    rhs=xh0[:, ct, qc * 512:(qc + 1) * 512],
                    start=(ct == 0), stop=(ct == 3))
            nc.vector.tensor_copy(qT[ot][:, qc * 512:(qc + 1) * 512], ps[:])

        def proj_chunk(qc, ot, evac_scalar=False):
            qh, qr = qc // 2, (qc % 2) * 512
            ps = pav.tile([128, 512], F32, tag="av", name="ps_proj")
            for ct in range(4):
                nc.tensor.matmul(
                    ps[:], lhsT=wp_sb[:, ct, ot * 128:(ot + 1) * 128],
                    rhs=vout[ct][qh][:, qr:qr + 512],
                    start=(ct == 0), stop=(ct == 3))
            yb = pyb.tile([128, 512], BF16, tag="yb", name="yb")
            if evac_scalar:   # tail: ACT is idle after the last exp
                nc.scalar.activation(yb[:], ps[:], AF.Identity,
                                     bias=bp_sb[:, ot:ot + 1])
            else:
                nc.vector.tensor_scalar_add(yb[:], ps[:], bp_sb[:, ot:ot + 1])
            nc.sync.dma_start(
                out=yt[ot * 128:(ot + 1) * 128, qc * 512:(qc + 1) * 512],
                in_=yb[:])

        def score_pair(hp, qc, kt):
            kTh = kT0 if kt < 4 else kT1
            ksl = slice((kt % 4) * 128, (kt % 4) * 128 + 128)
            qsl = slice(qc * 512, (qc + 1) * 512)
            sc = pp.tile([128, 1024], F32, tag="sc", name="sc")
            for sub in range(2):
                rr = sub * 64
                nc.tensor.matmul(
                    sc[:, sub * 512:(sub + 1) * 512],
                    lhsT=kTh[hp][rr:rr + 64, ksl],
                    rhs=qT[hp][rr:rr + 64, qsl],
                    start=True, stop=True)
            ex = pexp.tile([128, 1024], BF16, tag="ex", name="ex")
            nc.scalar.activation(ex[:], sc[:], AF.Exp, scale=SCALE)
            return ex

        # global software pipeline: the score/exp stream leads the av
        # stream by one kt slot and runs continuously ACROSS unit
        # boundaries, so the exp engine never waits at a boundary
        units = [(hp, qc) for qc in range(4) for hp in range(4)]
        blocks = [(i, 0, 8) for i in range(16)]
        seq = [(i, kt) for (i, b, e) in blocks for kt in range(b, e)]
        av_tiles = {}
        ex_p = score_pair(units[seq[0][0]][0], units[seq[0][0]][1], seq[0][1])
        for n, (idx, kt) in enumerate(seq):
            hp, qc = units[idx]
            qh, qr = qc // 2, (qc % 2) * 512
            if kt == 0:
                av_tiles[idx] = (
                    pav.tile([128, 512], F32, tag="av", name="av0"),
                    pav.tile([128, 512], F32, tag="av", name="av1"))
            av0, av1 = av_tiles[idx]
            ex = ex_p
            if n + 1 < len(seq):
                ni, nkt = seq[n + 1]
                ex_p = score_pair(units[ni][0], units[ni][1], nkt)
            nc.tensor.matmul(av0[:], lhsT=v_sb[kt][:, 2 * hp, :],
                             rhs=ex[:, 0:512],
                             start=(kt == 0), stop=(kt == 7))
            nc.tensor.matmul(av1[:], lhsT=v_sb[kt][:, 2 * hp + 1, :],
                             rhs=ex[:, 512:1024],
                             start=(kt == 0), stop=(kt == 7))
            if kt == 2 and qc in (1, 2):
                tc.cur_priority += 14   # filler work: schedule after the
                q_chunk(hp, qc + 1)     # score/exp stream
                tc.cur_priority -= 14
            if kt == 5 and qc > 0:
                tc.cur_priority += 14
                proj_chunk(qc - 1, hp)
                tc.cur_priority -= 14
            if kt == 7:
                for h, av in ((0, av0), (1, av1)):
                    rbc = prb.tile([64, 512], F32, tag="rbc", name="rbc")
                    nc.vector.reciprocal_approx_fast(out=rbc[:],
                                                     in_=av[0:64, :])
                    nc.vector.tensor_mul(
                        vout[hp][qh][h * 64:(h + 1) * 64, qr:qr + 512],
                        av[64:128, :], rbc[:])
        for ot in range(4):
            proj_chunk(3, ot, evac_scalar=True)


_CACHE = {}


def _build():
    if "nc" in _CACHE:
        return _CACHE["nc"]
    nc = bacc.Bacc("TRN2", target_bir_lowering=False, debug=False, num_devices=8)
    io = (
        nc.dram_tensor("xq", [128, 4, NQ], BF16, kind="ExternalInput").ap(),
        nc.dram_tensor("xo", [128, 4, NQ], BF16, kind="ExternalInput").ap(),
        nc.dram_tensor("w2", [4, 128, 4, C], BF16, kind="ExternalInput").ap(),
        nc.dram_tensor("wq", [128, 4, C], BF16, kind="ExternalInput").ap(),
        nc.dram_tensor("wkg", [128, 4, C], BF16, kind="ExternalInput").ap(),
        nc.dram_tensor("wkg2", [2, C], BF16, kind="ExternalInput").ap(),
        nc.dram_tensor("wvg", [128, 4, C], BF16, kind="ExternalInput").ap(),
        nc.dram_tensor("wvg2", [2, C], BF16, kind="ExternalInput").ap(),
        nc.dram_tensor("wp", [128, 4, C], BF16, kind="ExternalInput").ap(),
        nc.dram_tensor("bsr_t", [128, 4], F32, kind="ExternalInput").ap(),
        nc.dram_tensor("bp_t", [128, 4], F32, kind="ExternalInput").ap(),
        nc.dram_tensor("ones_row", [1, NKV], BF16, kind="ExternalInput").ap(),
        nc.dram_tensor("ones_c1", [128, 1], F32R, kind="ExternalInput").ap(),
        nc.dram_tensor("yt", [C, NQ], BF16, kind="ExternalOutput").ap(),
    )
    with tile.TileContext(nc) as tc, ExitStack() as ctx:
        _emit(nc, tc, ctx, io)
    nc.compile()
    _CACHE["nc"] = nc
    return nc


def _prep_inputs(x, Wq, Wkv, Wsr, bsr, ln_g, ln_b, Wp, bp):
    x = np.asarray(x, np.float32)
    Wq = np.asarray(Wq, np.float32)
    Wkv = np.asarray(Wkv, np.float32)
    Wsr = np.asarray(Wsr, np.float32)
    bsr = np.asarray(bsr, np.float32)
    ln_g = np.asarray(ln_g, np.float32)
    ln_b = np.asarray(ln_b, np.float32)
    Wp = np.asarray(Wp, np.float32)
    bp = np.asarray(bp, np.float32)

    def pct(a):
        return np.ascontiguousarray(a.reshape(4, 128, -1).transpose(1, 0, 2))

    # w2[ct][128(part=in-ch), didj, out-ch]
    w2 = np.ascontiguousarray(
        Wsr.transpose(2, 3, 1, 0).reshape(4, 4, 128, C).transpose(1, 2, 0, 3)
        .astype(BF))
    wq = pct(Wq.T.astype(BF))
    Wk, Wv = Wkv[:C], Wkv[C:]

    def ext(W):
        main = pct((W * ln_g[None, :]).T.astype(BF))                    # [p, ct, o]
        rows = np.stack([W @ ln_g, W @ ln_b]).astype(BF)                # [2, o]
        return main, np.ascontiguousarray(rows)

    wkg, wkg2 = ext(Wk)
    wvg, wvg2 = ext(Wv)
    wp = pct(Wp.T.astype(BF))
    bsr_t = np.ascontiguousarray(bsr.reshape(4, 128).T)
    bp_t = np.ascontiguousarray(bp.reshape(4, 128).T)

    shared = dict(w2=w2, wq=wq, wkg=wkg, wkg2=wkg2, wvg=wvg, wvg2=wvg2,
                  wp=wp, bsr_t=bsr_t, bp_t=bp_t,
                  ones_row=np.ones((1, NKV), BF),
                  ones_c1=np.ones((128, 1), np.float32))
    in_maps = []
    for core in range(8):
        b, half = core // 2, core % 2
        xT = x[b].T.astype(BF)                # [C, NTOK]
        m = dict(shared)
        m["xq"] = pct(xT[:, half * NQ:(half + 1) * NQ])
        m["xo"] = pct(xT[:, (1 - half) * NQ:(2 - half) * NQ])
        in_maps.append(m)
    return in_maps


def kernel(x, H, W, Wq, Wkv, Wsr, bsr, ln_g, ln_b, Wp, bp, _trace=False):
    nc = _build()
    in_maps = _prep_inputs(x, Wq, Wkv, Wsr, bsr, ln_g, ln_b, Wp, bp)
    res = run_bass_kernel_spmd(nc, in_maps, list(range(8)), trace=_trace)
    y = np.empty((B, NTOK, C), np.float32)
    for core in range(8):
        b, half = core // 2, core % 2
        y[b, half * NQ:(half + 1) * NQ, :] = \
            res.results[core]["yt"].astype(np.float32).T
    kernel._last_result = res
    return y
